# revision 9
# baseline (speedup 1.0000x reference)
"""BinaryMLP (nn_BinaryMLP_91276644974884) on 8 TRN2 NeuronCores.

Reference network (B=32768, D=784, H1=H2=4096, C=10):
    h  = x @ W1.T + b1                    # fc1
    h  = BN1(prelu(h, a1)) (batch stats)
    h  = sign(h) @ sign(W2).T             # fc2, binary GEMM
    h  = BN2(prelu(h, a2))
    o  = log_softmax(h @ W3.T + b3)

Strategy: data-parallel over batch (4096 rows/core), everything computed in a
transposed [features, batch] layout so BatchNorm stats are free-axis
reductions.  fc1 runs in float32r (measured err std ~1.5e-4 — sign-safe);
fc2 in bf16 which is EXACT for +-1 inputs (fp32 PSUM accumulate); the two
BatchNorm full-batch statistics are tiny [128, 64] AllReduces across the 8
cores.  prelu+row-sums fuse into one ScalarE Prelu pass; sign+BN fuse into
one ScalarE Sign pass (out = Sign(scale*p + bias)).

Host-side prep (free - not on device critical path): transposes/blocked
weight layouts, sign(W2) cast to bf16, bias rows folded into fc1's
contraction dim (row 784 of x^T is ones, of W1^T is b1).
"""

import numpy as np
import ml_dtypes

import concourse.bass as bass
import concourse.tile as tile
from concourse import bacc, mybir
from concourse.bass_utils import run_bass_kernel_spmd

F32 = mybir.dt.float32
F32R = mybir.dt.float32r
F16 = mybir.dt.float16
BF16 = mybir.dt.bfloat16
AF = mybir.ActivationFunctionType
ALU = mybir.AluOpType

NCORES = 8
B = 32768
BS = B // NCORES          # 4096 batch rows per core
D = 784
KP = 896                  # fc1 contraction: 784 features + ones row + zero pad
KC1 = 3 * (KP // 128)     # 21: fp16 hi/lo split, [xh; xh; xl] vs [whs; wls; wh]
FSPLIT = 2048.0           # 2^11 hi/lo split scale
H1 = 4096
H2 = 4096
MT = 32                   # 4096 / 128 feature tiles
C = 10
SUPER = 2048              # batch super-chunk resident in SBUF during fc2
NSUP = BS // SUPER        # 2
NNS = SUPER // 512        # 4 512-col chunks per super
NB = BS // 512            # 8 512-col chunks per core
EPS = 1e-5


def build_program(debug=False):
    nc = bacc.Bacc("TRN2", target_bir_lowering=False, debug=False,
                   num_devices=NCORES)

    xT = nc.declare_dram_parameter("xT", [128, KC1, BS], F16, isOutput=False)
    w1 = nc.declare_dram_parameter("w1", [MT, 128, KC1, 128], F16, isOutput=False)
    w2 = nc.declare_dram_parameter("w2", [MT, 128, MT, 128], BF16, isOutput=False)
    w3 = nc.declare_dram_parameter("w3", [128, MT, C], BF16, isOutput=False)
    g1 = nc.declare_dram_parameter("g1", [128, MT], F32, isOutput=False)
    bt1 = nc.declare_dram_parameter("bt1", [128, MT], F32, isOutput=False)
    g2 = nc.declare_dram_parameter("g2", [128, MT], F32, isOutput=False)
    bt2 = nc.declare_dram_parameter("bt2", [128, MT], F32, isOutput=False)
    a1p = nc.declare_dram_parameter("a1p", [128, 1], F32, isOutput=False)
    a2p = nc.declare_dram_parameter("a2p", [128, 1], F32, isOutput=False)
    b3p = nc.declare_dram_parameter("b3p", [C, 1], F32, isOutput=False)
    eye = nc.declare_dram_parameter("eye", [C, C], F32, isOutput=False)
    out = nc.declare_dram_parameter("out", [BS, C], F32, isOutput=True)

    dbg = {}
    if debug:
        for nm, shp in [
            ("dbg_stats1", [128, 2 * MT]), ("dbg_red1", [128, 2 * MT]),
            ("dbg_scale1", [128, MT]), ("dbg_bias1", [128, MT]),
            ("dbg_red2", [128, 2 * MT]),
            ("dbg_p1", [128, 512]), ("dbg_p2", [128, 512]),
        ]:
            dbg[nm] = nc.declare_dram_parameter(nm, shp, F32, isOutput=True)

    with tile.TileContext(nc) as tc:
        with (
            tc.tile_pool(name="const", bufs=1) as const_pool,
            tc.tile_pool(name="stats", bufs=1) as stats_pool,
            tc.tile_pool(name="dram", bufs=1, space="DRAM") as dram_pool,
            tc.tile_pool(name="psmm", bufs=4, space="PSUM") as ps_mm,
        ):
            # ---- persistent small tiles -------------------------------------
            g1_t = const_pool.tile([128, MT], F32, tag="g1")
            bt1_t = const_pool.tile([128, MT], F32, tag="bt1")
            g2_t = const_pool.tile([128, MT], F32, tag="g2")
            bt2_t = const_pool.tile([128, MT], F32, tag="bt2")
            a1_t = const_pool.tile([128, 1], F32, tag="a1")
            a2_t = const_pool.tile([128, 1], F32, tag="a2")
            b3_t = const_pool.tile([C, 1], F32, tag="b3")
            eye_t = const_pool.tile([C, C], F32, tag="eye")
            w3_t = const_pool.tile([128, MT, C], BF16, tag="w3")
            for t, d in [(g1_t, g1), (bt1_t, bt1), (g2_t, g2), (bt2_t, bt2),
                         (a1_t, a1p), (a2_t, a2p), (b3_t, b3p), (eye_t, eye),
                         (w3_t, w3)]:
                nc.sync.dma_start(t[:], d.ap())

            sums1 = stats_pool.tile([128, MT, NB], F32, tag="sums1")
            sq1 = stats_pool.tile([128, MT, NB], F32, tag="sq1")
            sums2 = stats_pool.tile([128, MT, NB], F32, tag="sums2")
            sq2 = stats_pool.tile([128, MT, NB], F32, tag="sq2")

            p1d = dram_pool.tile([MT, 128, BS], F32, tag="p1d")
            p2d = dram_pool.tile([MT, 128, BS], F16, tag="p2d")
            cc_in1 = dram_pool.tile([128, 2 * MT], F32, tag="cc_in1")
            cc_out1 = dram_pool.tile([128, 2 * MT], F32, tag="cc_out1")
            cc_in2 = dram_pool.tile([128, 2 * MT], F32, tag="cc_in2")
            cc_out2 = dram_pool.tile([128, 2 * MT], F32, tag="cc_out2")

            # ================= Phase 1: fc1 + prelu + stats ==================
            with (
                tc.tile_pool(name="xt", bufs=1) as xt_pool,
                tc.tile_pool(name="w1p", bufs=3) as w1_pool,
                tc.tile_pool(name="p1t", bufs=4) as p1_pool,
                tc.tile_pool(name="scr1", bufs=3) as scr_pool,
            ):
                xt_t = xt_pool.tile([128, KC1, BS], F16, tag="xt")
                for k in range(KC1):
                    nc.sync.dma_start(xt_t[:, k, :], xT.ap()[:, k, :])
                for m in range(MT):
                    w1_t = w1_pool.tile([128, KC1, 128], F16, tag="w1")
                    nc.sync.dma_start(w1_t[:], w1.ap()[m])
                    for n in range(NB):
                        ps = ps_mm.tile([128, 512], F32, tag="mm")
                        for k in range(KC1):
                            nc.tensor.matmul(
                                ps[:], w1_t[:, k, :],
                                xt_t[:, k, n * 512:(n + 1) * 512],
                                start=(k == 0), stop=(k == KC1 - 1),
                            )
                        p1_t = p1_pool.tile([128, 512], F32, tag="p1")
                        nc.scalar.activation(
                            p1_t[:], ps[:], AF.Prelu, alpha=a1_t[:],
                            scale=1.0 / FSPLIT,
                            accum_out=sums1[:, m, n:n + 1],
                        )
                        scr = scr_pool.tile([128, 512], F16, tag="scr")
                        nc.vector.scalar_tensor_tensor(
                            scr[:], p1_t[:], 0.0, p1_t[:], ALU.add, ALU.mult,
                            accum_out=sq1[:, m, n:n + 1],
                        )
                        nc.sync.dma_start(
                            p1d[m, :, n * 512:(n + 1) * 512], p1_t[:]
                        )

            # ---- BN1 stats: local reduce, AllReduce, finalize ---------------
            def bn_finalize(sums, sq, cc_in, cc_out, g_t, bt_t, tag):
                cat = stats_pool.tile([128, 2 * MT], F32, tag=f"cat{tag}")
                nc.vector.reduce_sum(cat[:, 0:MT], sums[:], axis=mybir.AxisListType.X)
                nc.vector.reduce_sum(cat[:, MT:], sq[:], axis=mybir.AxisListType.X)
                nc.sync.dma_start(cc_in[:], cat[:])
                nc.gpsimd.collective_compute(
                    "AllReduce", ALU.add,
                    replica_groups=[list(range(NCORES))],
                    ins=[cc_in.opt()], outs=[cc_out.opt()],
                )
                red = stats_pool.tile([128, 2 * MT], F32, tag=f"red{tag}")
                nc.sync.dma_start(red[:], cc_out[:])
                mu = stats_pool.tile([128, MT], F32, tag=f"mu{tag}")
                nc.vector.tensor_scalar_mul(mu[:], red[:, 0:MT], 1.0 / B)
                var = stats_pool.tile([128, MT], F32, tag=f"var{tag}")
                # var = E[p^2] - mu^2 + EPS  (fold the +EPS in here)
                nc.vector.tensor_mul(var[:], mu[:], mu[:])
                nc.vector.scalar_tensor_tensor(
                    var[:], red[:, MT:], 1.0 / B, var[:], ALU.mult, ALU.subtract,
                )
                nc.vector.tensor_scalar_add(var[:], var[:], EPS)
                rinv = stats_pool.tile([128, MT], F32, tag=f"rinv{tag}")
                nc.vector.reciprocal(rinv[:], var[:])
                r = stats_pool.tile([128, MT], F32, tag=f"r{tag}")
                nc.scalar.activation(r[:], rinv[:], AF.Sqrt)
                scale = stats_pool.tile([128, MT], F32, tag=f"scale{tag}")
                nc.vector.tensor_mul(scale[:], g_t[:], r[:])
                bias = stats_pool.tile([128, MT], F32, tag=f"bias{tag}")
                nc.vector.tensor_mul(bias[:], mu[:], scale[:])
                nc.vector.tensor_sub(bias[:], bt_t[:], bias[:])
                return cat, red, scale, bias

            cat1, red1, scale1, bias1 = bn_finalize(
                sums1, sq1, cc_in1, cc_out1, g1_t, bt1_t, "1"
            )

            # ================= Phase 2: sign + fc2 + prelu + stats ===========
            for s in range(NSUP):
                with (
                    tc.tile_pool(name=f"s1_{s}", bufs=1) as s1_pool,
                    tc.tile_pool(name=f"pin{s}", bufs=3) as pin_pool,
                    tc.tile_pool(name=f"w2_{s}", bufs=3) as w2_pool,
                    tc.tile_pool(name=f"p2_{s}", bufs=4) as p2_pool,
                    tc.tile_pool(name=f"sc2{s}", bufs=3) as scr2_pool,
                ):
                    lo = s * SUPER
                    s1_t = s1_pool.tile([128, MT, SUPER], BF16, tag="s1")
                    for m in range(MT):
                        pin = pin_pool.tile([128, SUPER], F32, tag="pin")
                        nc.sync.dma_start(pin[:], p1d[m, :, lo:lo + SUPER])
                        nc.scalar.activation(
                            s1_t[:, m, :], pin[:], AF.Sign,
                            bias=bias1[:, m:m + 1], scale=scale1[:, m:m + 1],
                        )
                    for m in range(MT):
                        w2_t = w2_pool.tile([128, MT, 128], BF16, tag="w2")
                        nc.sync.dma_start(w2_t[:], w2.ap()[m])
                        for j in range(NNS):
                            ps = ps_mm.tile([128, 512], F32, tag="mm")
                            for k in range(MT):
                                nc.tensor.matmul(
                                    ps[:], w2_t[:, k, :],
                                    s1_t[:, k, j * 512:(j + 1) * 512],
                                    start=(k == 0), stop=(k == MT - 1),
                                )
                            n_g = s * NNS + j
                            p2_t = p2_pool.tile([128, 512], F16, tag="p2")
                            nc.scalar.activation(
                                p2_t[:], ps[:], AF.Prelu, alpha=a2_t[:],
                                accum_out=sums2[:, m, n_g:n_g + 1],
                            )
                            scr = scr2_pool.tile([128, 512], F16, tag="scr2")
                            nc.vector.scalar_tensor_tensor(
                                scr[:], p2_t[:], 0.0, p2_t[:], ALU.add, ALU.mult,
                                accum_out=sq2[:, m, n_g:n_g + 1],
                            )
                            nc.sync.dma_start(
                                p2d[m, :, lo + j * 512:lo + (j + 1) * 512],
                                p2_t[:],
                            )

            cat2, red2, scale2, bias2 = bn_finalize(
                sums2, sq2, cc_in2, cc_out2, g2_t, bt2_t, "2"
            )

            # ================= Phase 3: BN2 + fc3 + log_softmax ==============
            with (
                tc.tile_pool(name="qp", bufs=4) as q_pool,
                tc.tile_pool(name="lg", bufs=2) as lg_pool,
                tc.tile_pool(name="ps3", bufs=2, space="PSUM") as ps3_pool,
                tc.tile_pool(name="pst", bufs=2, space="PSUM") as pst_pool,
                tc.tile_pool(name="sm", bufs=4) as sm_pool,
                tc.tile_pool(name="op", bufs=4) as out_pool,
            ):
                for n in range(NB):
                    pl = ps3_pool.tile([C, 512], F32, tag="pl")
                    for k in range(MT):
                        qin = q_pool.tile([128, 512], F16, tag="qin")
                        nc.sync.dma_start(
                            qin[:], p2d[k, :, n * 512:(n + 1) * 512]
                        )
                        q = q_pool.tile([128, 512], BF16, tag="q")
                        nc.scalar.activation(
                            q[:], qin[:], AF.Identity,
                            bias=bias2[:, k:k + 1], scale=scale2[:, k:k + 1],
                        )
                        nc.tensor.matmul(
                            pl[:], w3_t[:, k, :], q[:],
                            start=(k == 0), stop=(k == MT - 1),
                        )
                    lg = lg_pool.tile([C, 512], F32, tag="lg")
                    nc.scalar.activation(lg[:], pl[:], AF.Identity, bias=b3_t[:])
                    for j in range(4):
                        pt = pst_pool.tile([128, C], F32, tag="pt")
                        nc.tensor.transpose(
                            pt[:], lg[:, j * 128:(j + 1) * 128], eye_t[:]
                        )
                        mx = sm_pool.tile([128, 1], F32, tag="mx")
                        nc.vector.reduce_max(
                            mx[:], pt[:], axis=mybir.AxisListType.X, negate=True
                        )
                        ex = sm_pool.tile([128, C], F32, tag="ex")
                        se = sm_pool.tile([128, 1], F32, tag="se")
                        nc.scalar.activation(
                            ex[:], pt[:], AF.Exp, bias=mx[:], accum_out=se[:]
                        )
                        ln = sm_pool.tile([128, 1], F32, tag="ln")
                        nc.scalar.activation(ln[:], se[:], AF.Ln)
                        adj = sm_pool.tile([128, 1], F32, tag="adj")
                        nc.vector.tensor_sub(adj[:], mx[:], ln[:])
                        ot = out_pool.tile([128, C], F32, tag="ot")
                        nc.vector.tensor_scalar(
                            ot[:], pt[:], adj[:], None, ALU.add
                        )
                        nc.sync.dma_start(
                            out.ap()[n * 512 + j * 128:n * 512 + (j + 1) * 128, :],
                            ot[:],
                        )

            if debug:
                for nm, t in [("dbg_stats1", cat1), ("dbg_red1", red1),
                              ("dbg_scale1", scale1), ("dbg_bias1", bias1),
                              ("dbg_red2", red2)]:
                    nc.sync.dma_start(dbg[nm].ap(), t[:])
                d1f = stats_pool.tile([128, 512], F32, tag="d1f")
                nc.sync.dma_start(d1f[:], p1d[0, :, 0:512])
                nc.sync.dma_start(dbg["dbg_p1"].ap(), d1f[:])
                d2 = stats_pool.tile([128, 512], F16, tag="d2")
                nc.sync.dma_start(d2[:], p2d[0, :, 0:512])
                d2f = stats_pool.tile([128, 512], F32, tag="d2f")
                nc.vector.tensor_copy(d2f[:], d2[:])
                nc.sync.dma_start(dbg["dbg_p2"].ap(), d2f[:])

    nc.compile()
    return nc


def prep_inputs(x, W1, b1, a1, g1, beta1, W2, a2, g2, beta2, W3, b3):
    """Host-side layout prep. Returns per-core in_maps."""
    x = np.ascontiguousarray(np.asarray(x, np.float32))
    W1 = np.asarray(W1, np.float32)
    b1 = np.asarray(b1, np.float32)
    W2 = np.asarray(W2, np.float32)
    W3 = np.asarray(W3, np.float32)
    b3 = np.asarray(b3, np.float32)

    # fc1 operands with bias folded in as contraction row 784 (rows 785+ zero).
    # fp16 hi/lo split with 2^11 scaling, packed along K:
    #   XF = [xh; xh; xl*S],  WF = [wh*S; wl*S; wh]  ->  psum = S * h1
    # where v = vh + vl exactly captures ~22 mantissa bits.  The bias row uses
    # x-side 32.0 / w-side b1/32 to keep w*S within fp16 range.
    S = np.float32(FSPLIT)
    xT_aug = np.zeros((KP, B), np.float32)
    xT_aug[0:D] = x.T
    xT_aug[D] = 32.0
    w1T_aug = np.zeros((KP, H1), np.float32)
    w1T_aug[0:D] = W1.T
    w1T_aug[D] = b1 / 32.0

    xh = xT_aug.astype(np.float16)
    xl = ((xT_aug - xh.astype(np.float32)) * S).astype(np.float16)
    wh = w1T_aug.astype(np.float16)
    whs = (w1T_aug * S).astype(np.float16)
    wls = ((w1T_aug - wh.astype(np.float32)) * S).astype(np.float16)
    xF = np.concatenate([xh, xh, xl], axis=0)        # [3*KP, B] fp16
    wF = np.concatenate([whs, wls, wh], axis=0)      # [3*KP, H1] fp16
    w1_blk = np.ascontiguousarray(
        wF.reshape(KC1, 128, MT, 128).transpose(2, 1, 0, 3)
    )

    sW2T = np.where(W2 >= 0, np.float32(1), np.float32(-1)).T
    w2_blk = np.ascontiguousarray(
        sW2T.reshape(MT, 128, MT, 128).transpose(2, 1, 0, 3)
    ).astype(ml_dtypes.bfloat16)

    w3_blk = np.ascontiguousarray(
        W3.T.reshape(MT, 128, C).transpose(1, 0, 2)
    ).astype(ml_dtypes.bfloat16)

    def feat_layout(v):
        return np.ascontiguousarray(np.asarray(v, np.float32).reshape(MT, 128).T)

    shared = dict(
        w1=w1_blk, w2=w2_blk, w3=w3_blk,
        g1=feat_layout(g1), bt1=feat_layout(beta1),
        g2=feat_layout(g2), bt2=feat_layout(beta2),
        a1p=np.full((128, 1), np.float32(a1), np.float32),
        a2p=np.full((128, 1), np.float32(a2), np.float32),
        b3p=b3.reshape(C, 1).astype(np.float32),
        eye=np.eye(C, dtype=np.float32),
    )
    in_maps = []
    for c in range(NCORES):
        sl = xF[:, c * BS:(c + 1) * BS]
        xs = np.ascontiguousarray(sl.reshape(KC1, 128, BS).transpose(1, 0, 2))
        in_maps.append(dict(shared, xT=xs))
    return in_maps


_NC_CACHE = {}


def run(inputs, debug=False, trace=False):
    key = (debug,)
    if key not in _NC_CACHE:
        _NC_CACHE[key] = build_program(debug=debug)
    nc = _NC_CACHE[key]
    in_maps = prep_inputs(**inputs)
    res = run_bass_kernel_spmd(
        nc, in_maps, core_ids=list(range(NCORES)), trace=trace
    )
    outs = np.concatenate([res.results[c]["out"] for c in range(NCORES)], axis=0)
    return outs, res


def kernel(**inputs):
    out, _ = run(inputs)
    return out


# revision 10
# speedup vs baseline: 1.0367x; 1.0367x over previous
"""BinaryMLP (nn_BinaryMLP_91276644974884) on 8 TRN2 NeuronCores.

Reference network (B=32768, D=784, H1=H2=4096, C=10):
    h  = x @ W1.T + b1                    # fc1
    h  = BN1(prelu(h, a1)) (batch stats)
    h  = sign(h) @ sign(W2).T             # fc2, binary GEMM
    h  = BN2(prelu(h, a2))
    o  = log_softmax(h @ W3.T + b3)

Strategy: data-parallel over batch (4096 rows/core), everything computed in a
transposed [features, batch] layout so BatchNorm stats are free-axis
reductions.  fc1 runs in float32r (measured err std ~1.5e-4 — sign-safe);
fc2 in bf16 which is EXACT for +-1 inputs (fp32 PSUM accumulate); the two
BatchNorm full-batch statistics are tiny [128, 64] AllReduces across the 8
cores.  prelu+row-sums fuse into one ScalarE Prelu pass; sign+BN fuse into
one ScalarE Sign pass (out = Sign(scale*p + bias)).

Host-side prep (free - not on device critical path): transposes/blocked
weight layouts, sign(W2) cast to bf16, bias rows folded into fc1's
contraction dim (row 784 of x^T is ones, of W1^T is b1).
"""

import numpy as np
import ml_dtypes

import concourse.bass as bass
import concourse.tile as tile
from concourse import bacc, mybir
from concourse.bass_utils import run_bass_kernel_spmd

F32 = mybir.dt.float32
F32R = mybir.dt.float32r
F16 = mybir.dt.float16
BF16 = mybir.dt.bfloat16
AF = mybir.ActivationFunctionType
ALU = mybir.AluOpType

NCORES = 8
B = 32768
BS = B // NCORES          # 4096 batch rows per core
D = 784
KP = 896                  # fc1 contraction: 784 features + ones row + zero pad
KC1 = 3 * (KP // 128)     # 21: fp16 hi/lo split, [xh; xh; xl] vs [whs; wls; wh]
FSPLIT = 2048.0           # 2^11 hi/lo split scale
H1 = 4096
H2 = 4096
MT = 32                   # 4096 / 128 feature tiles
C = 10
SUPER = 2048              # batch super-chunk resident in SBUF during fc2
NSUP = BS // SUPER        # 2
NNS = SUPER // 512        # 4 512-col chunks per super
NB = BS // 512            # 8 512-col chunks per core
EPS = 1e-5


def build_program(debug=False):
    nc = bacc.Bacc("TRN2", target_bir_lowering=False, debug=False,
                   num_devices=NCORES)

    xT = nc.declare_dram_parameter("xT", [128, KC1, BS], F16, isOutput=False)
    w1 = nc.declare_dram_parameter("w1", [MT, 128, KC1, 128], F16, isOutput=False)
    w2 = nc.declare_dram_parameter("w2", [MT, 128, MT, 128], BF16, isOutput=False)
    w3 = nc.declare_dram_parameter("w3", [128, MT, C], F16, isOutput=False)
    g1 = nc.declare_dram_parameter("g1", [128, MT], F32, isOutput=False)
    bt1 = nc.declare_dram_parameter("bt1", [128, MT], F32, isOutput=False)
    g2 = nc.declare_dram_parameter("g2", [128, MT], F32, isOutput=False)
    bt2 = nc.declare_dram_parameter("bt2", [128, MT], F32, isOutput=False)
    a1p = nc.declare_dram_parameter("a1p", [128, 1], F32, isOutput=False)
    a2p = nc.declare_dram_parameter("a2p", [128, 1], F32, isOutput=False)
    b3p = nc.declare_dram_parameter("b3p", [C, 1], F32, isOutput=False)
    eye = nc.declare_dram_parameter("eye", [C, C], F32, isOutput=False)
    out = nc.declare_dram_parameter("out", [BS, C], F32, isOutput=True)

    dbg = {}
    if debug:
        for nm, shp in [
            ("dbg_stats1", [128, 2 * MT]), ("dbg_red1", [128, 2 * MT]),
            ("dbg_scale1", [128, MT]), ("dbg_bias1", [128, MT]),
            ("dbg_red2", [128, 2 * MT]),
            ("dbg_p1", [128, 512]), ("dbg_p2", [128, 512]),
        ]:
            dbg[nm] = nc.declare_dram_parameter(nm, shp, F32, isOutput=True)

    with tile.TileContext(nc) as tc:
        with (
            tc.tile_pool(name="const", bufs=1) as const_pool,
            tc.tile_pool(name="stats", bufs=1) as stats_pool,
            tc.tile_pool(name="dram", bufs=1, space="DRAM") as dram_pool,
            tc.tile_pool(name="psmm", bufs=4, space="PSUM") as ps_mm,
        ):
            # ---- persistent small tiles -------------------------------------
            g1_t = const_pool.tile([128, MT], F32, tag="g1")
            bt1_t = const_pool.tile([128, MT], F32, tag="bt1")
            g2_t = const_pool.tile([128, MT], F32, tag="g2")
            bt2_t = const_pool.tile([128, MT], F32, tag="bt2")
            a1_t = const_pool.tile([128, 1], F32, tag="a1")
            a2_t = const_pool.tile([128, 1], F32, tag="a2")
            b3_t = const_pool.tile([C, 1], F32, tag="b3")
            eye_t = const_pool.tile([C, C], F32, tag="eye")
            w3_t = const_pool.tile([128, MT, C], F16, tag="w3")
            for t, d in [(g1_t, g1), (bt1_t, bt1), (g2_t, g2), (bt2_t, bt2),
                         (a1_t, a1p), (a2_t, a2p), (b3_t, b3p), (eye_t, eye),
                         (w3_t, w3)]:
                nc.sync.dma_start(t[:], d.ap())

            sums1 = stats_pool.tile([128, MT, NB], F32, tag="sums1")
            sq1 = stats_pool.tile([128, MT, NB], F32, tag="sq1")
            sums2 = stats_pool.tile([128, MT, NB], F32, tag="sums2")
            sq2 = stats_pool.tile([128, MT, NB], F32, tag="sq2")

            p1d = dram_pool.tile([MT, 128, BS], F32, tag="p1d")
            p2d = dram_pool.tile([MT, 128, BS], F16, tag="p2d")
            cc_in1 = dram_pool.tile([128, 2 * MT], F32, tag="cc_in1")
            cc_out1 = dram_pool.tile([128, 2 * MT], F32, tag="cc_out1")
            cc_in2 = dram_pool.tile([128, 2 * MT], F32, tag="cc_in2")
            cc_out2 = dram_pool.tile([128, 2 * MT], F32, tag="cc_out2")

            # ================= Phase 1: fc1 + prelu + stats ==================
            with (
                tc.tile_pool(name="xt", bufs=1) as xt_pool,
                tc.tile_pool(name="w1p", bufs=3) as w1_pool,
                tc.tile_pool(name="p1t", bufs=4) as p1_pool,
                tc.tile_pool(name="scr1", bufs=3) as scr_pool,
            ):
                xt_t = xt_pool.tile([128, KC1, BS], F16, tag="xt")
                for k in range(KC1):
                    nc.sync.dma_start(xt_t[:, k, :], xT.ap()[:, k, :])
                for m in range(MT):
                    w1_t = w1_pool.tile([128, KC1, 128], F16, tag="w1")
                    nc.sync.dma_start(w1_t[:], w1.ap()[m])
                    for n in range(NB):
                        ps = ps_mm.tile([128, 512], F32, tag="mm")
                        for k in range(KC1):
                            nc.tensor.matmul(
                                ps[:], w1_t[:, k, :],
                                xt_t[:, k, n * 512:(n + 1) * 512],
                                start=(k == 0), stop=(k == KC1 - 1),
                            )
                        p1_t = p1_pool.tile([128, 512], F32, tag="p1")
                        nc.scalar.activation(
                            p1_t[:], ps[:], AF.Prelu, alpha=a1_t[:],
                            scale=1.0 / FSPLIT,
                            accum_out=sums1[:, m, n:n + 1],
                        )
                        scr = scr_pool.tile([128, 512], F16, tag="scr")
                        nc.vector.scalar_tensor_tensor(
                            scr[:], p1_t[:], 0.0, p1_t[:], ALU.add, ALU.mult,
                            accum_out=sq1[:, m, n:n + 1],
                        )
                        nc.sync.dma_start(
                            p1d[m, :, n * 512:(n + 1) * 512], p1_t[:]
                        )

            # ---- BN1 stats: local reduce, AllReduce, finalize ---------------
            def bn_finalize(sums, sq, cc_in, cc_out, g_t, bt_t, tag):
                cat = stats_pool.tile([128, 2 * MT], F32, tag=f"cat{tag}")
                nc.vector.reduce_sum(cat[:, 0:MT], sums[:], axis=mybir.AxisListType.X)
                nc.vector.reduce_sum(cat[:, MT:], sq[:], axis=mybir.AxisListType.X)
                nc.sync.dma_start(cc_in[:], cat[:])
                nc.gpsimd.collective_compute(
                    "AllReduce", ALU.add,
                    replica_groups=[list(range(NCORES))],
                    ins=[cc_in.opt()], outs=[cc_out.opt()],
                )
                red = stats_pool.tile([128, 2 * MT], F32, tag=f"red{tag}")
                nc.sync.dma_start(red[:], cc_out[:])
                mu = stats_pool.tile([128, MT], F32, tag=f"mu{tag}")
                nc.vector.tensor_scalar_mul(mu[:], red[:, 0:MT], 1.0 / B)
                var = stats_pool.tile([128, MT], F32, tag=f"var{tag}")
                # var = E[p^2] - mu^2 + EPS  (fold the +EPS in here)
                nc.vector.tensor_mul(var[:], mu[:], mu[:])
                nc.vector.scalar_tensor_tensor(
                    var[:], red[:, MT:], 1.0 / B, var[:], ALU.mult, ALU.subtract,
                )
                nc.vector.tensor_scalar_add(var[:], var[:], EPS)
                rinv = stats_pool.tile([128, MT], F32, tag=f"rinv{tag}")
                nc.vector.reciprocal(rinv[:], var[:])
                r = stats_pool.tile([128, MT], F32, tag=f"r{tag}")
                nc.scalar.activation(r[:], rinv[:], AF.Sqrt)
                scale = stats_pool.tile([128, MT], F32, tag=f"scale{tag}")
                nc.vector.tensor_mul(scale[:], g_t[:], r[:])
                bias = stats_pool.tile([128, MT], F32, tag=f"bias{tag}")
                nc.vector.tensor_mul(bias[:], mu[:], scale[:])
                nc.vector.tensor_sub(bias[:], bt_t[:], bias[:])
                return cat, red, scale, bias

            cat1, red1, scale1, bias1 = bn_finalize(
                sums1, sq1, cc_in1, cc_out1, g1_t, bt1_t, "1"
            )

            # ================= Phase 2: sign + fc2 + prelu + stats ===========
            for s in range(NSUP):
                with (
                    tc.tile_pool(name=f"s1_{s}", bufs=1) as s1_pool,
                    tc.tile_pool(name=f"pin{s}", bufs=3) as pin_pool,
                    tc.tile_pool(name=f"w2_{s}", bufs=3) as w2_pool,
                    tc.tile_pool(name=f"p2_{s}", bufs=4) as p2_pool,
                    tc.tile_pool(name=f"sc2{s}", bufs=3) as scr2_pool,
                ):
                    lo = s * SUPER
                    s1_t = s1_pool.tile([128, MT, SUPER], BF16, tag="s1")
                    for m in range(MT):
                        pin = pin_pool.tile([128, SUPER], F32, tag="pin")
                        nc.sync.dma_start(pin[:], p1d[m, :, lo:lo + SUPER])
                        nc.scalar.activation(
                            s1_t[:, m, :], pin[:], AF.Sign,
                            bias=bias1[:, m:m + 1], scale=scale1[:, m:m + 1],
                        )
                    for m in range(MT):
                        w2_t = w2_pool.tile([128, MT, 128], BF16, tag="w2")
                        nc.sync.dma_start(w2_t[:], w2.ap()[m])
                        for j in range(NNS):
                            ps = ps_mm.tile([128, 512], F32, tag="mm")
                            for k in range(MT):
                                nc.tensor.matmul(
                                    ps[:], w2_t[:, k, :],
                                    s1_t[:, k, j * 512:(j + 1) * 512],
                                    start=(k == 0), stop=(k == MT - 1),
                                )
                            n_g = s * NNS + j
                            p2_t = p2_pool.tile([128, 512], F16, tag="p2")
                            nc.scalar.activation(
                                p2_t[:], ps[:], AF.Prelu, alpha=a2_t[:],
                                accum_out=sums2[:, m, n_g:n_g + 1],
                            )
                            scr = scr2_pool.tile([128, 512], F16, tag="scr2")
                            nc.vector.scalar_tensor_tensor(
                                scr[:], p2_t[:], 0.0, p2_t[:], ALU.add, ALU.mult,
                                accum_out=sq2[:, m, n_g:n_g + 1],
                            )
                            nc.sync.dma_start(
                                p2d[m, :, lo + j * 512:lo + (j + 1) * 512],
                                p2_t[:],
                            )

            cat2, red2, scale2, bias2 = bn_finalize(
                sums2, sq2, cc_in2, cc_out2, g2_t, bt2_t, "2"
            )

            # ================= Phase 3: BN2 + fc3 + log_softmax ==============
            with (
                tc.tile_pool(name="qp", bufs=4) as q_pool,
                tc.tile_pool(name="lg", bufs=2) as lg_pool,
                tc.tile_pool(name="ps3", bufs=2, space="PSUM") as ps3_pool,
                tc.tile_pool(name="pst", bufs=2, space="PSUM") as pst_pool,
                tc.tile_pool(name="sm", bufs=4) as sm_pool,
                tc.tile_pool(name="op", bufs=4) as out_pool,
            ):
                for n in range(NB):
                    pl = ps3_pool.tile([C, 512], F32, tag="pl")
                    for k in range(MT):
                        qin = q_pool.tile([128, 512], F16, tag="qin")
                        nc.sync.dma_start(
                            qin[:], p2d[k, :, n * 512:(n + 1) * 512]
                        )
                        q = q_pool.tile([128, 512], F16, tag="q")
                        nc.scalar.activation(
                            q[:], qin[:], AF.Identity,
                            bias=bias2[:, k:k + 1], scale=scale2[:, k:k + 1],
                        )
                        nc.tensor.matmul(
                            pl[:], w3_t[:, k, :], q[:],
                            start=(k == 0), stop=(k == MT - 1),
                        )
                    lg = lg_pool.tile([C, 512], F32, tag="lg")
                    nc.scalar.activation(lg[:], pl[:], AF.Identity, bias=b3_t[:])
                    for j in range(4):
                        pt = pst_pool.tile([128, C], F32, tag="pt")
                        nc.tensor.transpose(
                            pt[:], lg[:, j * 128:(j + 1) * 128], eye_t[:]
                        )
                        mx = sm_pool.tile([128, 1], F32, tag="mx")
                        nc.vector.reduce_max(
                            mx[:], pt[:], axis=mybir.AxisListType.X, negate=True
                        )
                        ex = sm_pool.tile([128, C], F32, tag="ex")
                        se = sm_pool.tile([128, 1], F32, tag="se")
                        nc.scalar.activation(
                            ex[:], pt[:], AF.Exp, bias=mx[:], accum_out=se[:]
                        )
                        ln = sm_pool.tile([128, 1], F32, tag="ln")
                        nc.scalar.activation(ln[:], se[:], AF.Ln)
                        adj = sm_pool.tile([128, 1], F32, tag="adj")
                        nc.vector.tensor_sub(adj[:], mx[:], ln[:])
                        ot = out_pool.tile([128, C], F32, tag="ot")
                        nc.vector.tensor_scalar(
                            ot[:], pt[:], adj[:], None, ALU.add
                        )
                        nc.sync.dma_start(
                            out.ap()[n * 512 + j * 128:n * 512 + (j + 1) * 128, :],
                            ot[:],
                        )

            if debug:
                for nm, t in [("dbg_stats1", cat1), ("dbg_red1", red1),
                              ("dbg_scale1", scale1), ("dbg_bias1", bias1),
                              ("dbg_red2", red2)]:
                    nc.sync.dma_start(dbg[nm].ap(), t[:])
                d1f = stats_pool.tile([128, 512], F32, tag="d1f")
                nc.sync.dma_start(d1f[:], p1d[0, :, 0:512])
                nc.sync.dma_start(dbg["dbg_p1"].ap(), d1f[:])
                d2 = stats_pool.tile([128, 512], F16, tag="d2")
                nc.sync.dma_start(d2[:], p2d[0, :, 0:512])
                d2f = stats_pool.tile([128, 512], F32, tag="d2f")
                nc.vector.tensor_copy(d2f[:], d2[:])
                nc.sync.dma_start(dbg["dbg_p2"].ap(), d2f[:])

    nc.compile()
    return nc


def prep_inputs(x, W1, b1, a1, g1, beta1, W2, a2, g2, beta2, W3, b3):
    """Host-side layout prep. Returns per-core in_maps."""
    x = np.ascontiguousarray(np.asarray(x, np.float32))
    W1 = np.asarray(W1, np.float32)
    b1 = np.asarray(b1, np.float32)
    W2 = np.asarray(W2, np.float32)
    W3 = np.asarray(W3, np.float32)
    b3 = np.asarray(b3, np.float32)

    # fc1 operands with bias folded in as contraction row 784 (rows 785+ zero).
    # fp16 hi/lo split with 2^11 scaling, packed along K:
    #   XF = [xh; xh; xl*S],  WF = [wh*S; wl*S; wh]  ->  psum = S * h1
    # where v = vh + vl exactly captures ~22 mantissa bits.  The bias row uses
    # x-side 32.0 / w-side b1/32 to keep w*S within fp16 range.
    S = np.float32(FSPLIT)
    xT_aug = np.zeros((KP, B), np.float32)
    xT_aug[0:D] = x.T
    xT_aug[D] = 32.0
    w1T_aug = np.zeros((KP, H1), np.float32)
    w1T_aug[0:D] = W1.T
    w1T_aug[D] = b1 / 32.0

    xh = xT_aug.astype(np.float16)
    xl = ((xT_aug - xh.astype(np.float32)) * S).astype(np.float16)
    wh = w1T_aug.astype(np.float16)
    whs = (w1T_aug * S).astype(np.float16)
    wls = ((w1T_aug - wh.astype(np.float32)) * S).astype(np.float16)
    xF = np.concatenate([xh, xh, xl], axis=0)        # [3*KP, B] fp16
    wF = np.concatenate([whs, wls, wh], axis=0)      # [3*KP, H1] fp16
    w1_blk = np.ascontiguousarray(
        wF.reshape(KC1, 128, MT, 128).transpose(2, 1, 0, 3)
    )

    sW2T = np.where(W2 >= 0, np.float32(1), np.float32(-1)).T
    w2_blk = np.ascontiguousarray(
        sW2T.reshape(MT, 128, MT, 128).transpose(2, 1, 0, 3)
    ).astype(ml_dtypes.bfloat16)

    w3_blk = np.ascontiguousarray(
        W3.T.reshape(MT, 128, C).transpose(1, 0, 2)
    ).astype(np.float16)

    def feat_layout(v):
        return np.ascontiguousarray(np.asarray(v, np.float32).reshape(MT, 128).T)

    shared = dict(
        w1=w1_blk, w2=w2_blk, w3=w3_blk,
        g1=feat_layout(g1), bt1=feat_layout(beta1),
        g2=feat_layout(g2), bt2=feat_layout(beta2),
        a1p=np.full((128, 1), np.float32(a1), np.float32),
        a2p=np.full((128, 1), np.float32(a2), np.float32),
        b3p=b3.reshape(C, 1).astype(np.float32),
        eye=np.eye(C, dtype=np.float32),
    )
    in_maps = []
    for c in range(NCORES):
        sl = xF[:, c * BS:(c + 1) * BS]
        xs = np.ascontiguousarray(sl.reshape(KC1, 128, BS).transpose(1, 0, 2))
        in_maps.append(dict(shared, xT=xs))
    return in_maps


_NC_CACHE = {}


def run(inputs, debug=False, trace=False):
    key = (debug,)
    if key not in _NC_CACHE:
        _NC_CACHE[key] = build_program(debug=debug)
    nc = _NC_CACHE[key]
    in_maps = prep_inputs(**inputs)
    res = run_bass_kernel_spmd(
        nc, in_maps, core_ids=list(range(NCORES)), trace=trace
    )
    outs = np.concatenate([res.results[c]["out"] for c in range(NCORES)], axis=0)
    return outs, res


def kernel(**inputs):
    out, _ = run(inputs)
    return out


# revision 17
# speedup vs baseline: 1.4857x; 1.4331x over previous
"""BinaryMLP (nn_BinaryMLP_91276644974884) on 8 TRN2 NeuronCores.

Reference network (B=32768, D=784, H1=H2=4096, C=10):
    h  = x @ W1.T + b1                    # fc1
    h  = BN1(prelu(h, a1)) (batch stats)
    h  = sign(h) @ sign(W2).T             # fc2, binary GEMM
    h  = BN2(prelu(h, a2))
    o  = log_softmax(h @ W3.T + b3)

Strategy: data-parallel over batch (4096 rows/core), everything computed in a
transposed [features, batch] layout so BatchNorm stats are free-axis
reductions.  fc1 runs in float32r (measured err std ~1.5e-4 — sign-safe);
fc2 in bf16 which is EXACT for +-1 inputs (fp32 PSUM accumulate); the two
BatchNorm full-batch statistics are tiny [128, 64] AllReduces across the 8
cores.  prelu+row-sums fuse into one ScalarE Prelu pass; sign+BN fuse into
one ScalarE Sign pass (out = Sign(scale*p + bias)).

Host-side prep (free - not on device critical path): transposes/blocked
weight layouts, sign(W2) cast to bf16, bias rows folded into fc1's
contraction dim (row 784 of x^T is ones, of W1^T is b1).
"""

import numpy as np
import ml_dtypes

import concourse.bass as bass
import concourse.tile as tile
from concourse import bacc, mybir
from concourse.bass_utils import run_bass_kernel_spmd

F32 = mybir.dt.float32
F32R = mybir.dt.float32r
F16 = mybir.dt.float16
BF16 = mybir.dt.bfloat16
F8 = mybir.dt.float8e4
AF = mybir.ActivationFunctionType
ALU = mybir.AluOpType

NCORES = 8
B = 32768
BS = B // NCORES          # 4096 batch rows per core
D = 784
K1ROWS = 2 * (D + 1) + D  # 2354: [xh+bias; xh+bias; xl] tightly packed along K
KC1 = -(-K1ROWS // 128)   # 19 chunks (padded to 2432)
FSPLIT = 2048.0           # 2^11 hi/lo split scale
H1 = 4096
H2 = 4096
MT = 32                   # 4096 / 128 feature tiles
C = 10
SUPER = 2048              # batch super-chunk resident in SBUF during fc2
NSUP = BS // SUPER        # 2
NNS = SUPER // 512        # 4 512-col chunks per super
NB = BS // 512            # 8 512-col chunks per core
EPS = 1e-5


def build_program(debug=False):
    nc = bacc.Bacc("TRN2", target_bir_lowering=False, debug=False,
                   num_devices=NCORES)

    xT = nc.declare_dram_parameter("xT", [128, KC1, BS], F16, isOutput=False)
    w1 = nc.declare_dram_parameter("w1", [MT, 128, KC1, 128], F16, isOutput=False)
    w2 = nc.declare_dram_parameter("w2", [MT, 128, MT, 128], F8, isOutput=False)
    w3 = nc.declare_dram_parameter("w3", [128, MT, C], F16, isOutput=False)
    g1 = nc.declare_dram_parameter("g1", [128, MT], F32, isOutput=False)
    bt1 = nc.declare_dram_parameter("bt1", [128, MT], F32, isOutput=False)
    g2 = nc.declare_dram_parameter("g2", [128, MT], F32, isOutput=False)
    bt2 = nc.declare_dram_parameter("bt2", [128, MT], F32, isOutput=False)
    a1p = nc.declare_dram_parameter("a1p", [128, 1], F32, isOutput=False)
    a2p = nc.declare_dram_parameter("a2p", [128, 1], F32, isOutput=False)
    b3p = nc.declare_dram_parameter("b3p", [C, 1], F32, isOutput=False)
    eye = nc.declare_dram_parameter("eye", [C, C], F32, isOutput=False)
    out = nc.declare_dram_parameter("out", [BS, C], F32, isOutput=True)

    dbg = {}
    if debug:
        for nm, shp in [
            ("dbg_stats1", [128, 2 * MT]), ("dbg_red1", [128, 2 * MT]),
            ("dbg_scale1", [128, MT]), ("dbg_bias1", [128, MT]),
            ("dbg_red2", [128, 2 * MT]),
            ("dbg_p1", [128, 512]), ("dbg_p2", [128, 512]),
        ]:
            dbg[nm] = nc.declare_dram_parameter(nm, shp, F32, isOutput=True)

    with tile.TileContext(nc) as tc:
        with (
            tc.tile_pool(name="const", bufs=1) as const_pool,
            tc.tile_pool(name="stats", bufs=1) as stats_pool,
            tc.tile_pool(name="dram", bufs=1, space="DRAM") as dram_pool,
            tc.tile_pool(name="psmm", bufs=4, space="PSUM") as ps_mm,
        ):
            # ---- persistent small tiles -------------------------------------
            g1_t = const_pool.tile([128, MT], F32, tag="g1")
            bt1_t = const_pool.tile([128, MT], F32, tag="bt1")
            g2_t = const_pool.tile([128, MT], F32, tag="g2")
            bt2_t = const_pool.tile([128, MT], F32, tag="bt2")
            a1_t = const_pool.tile([128, 1], F32, tag="a1")
            a2_t = const_pool.tile([128, 1], F32, tag="a2")
            b3_t = const_pool.tile([C, 1], F32, tag="b3")
            eye_t = const_pool.tile([C, C], F32, tag="eye")
            w3_t = const_pool.tile([128, MT, C], F16, tag="w3")
            for t, d in [(g1_t, g1), (bt1_t, bt1), (g2_t, g2), (bt2_t, bt2),
                         (a1_t, a1p), (a2_t, a2p), (b3_t, b3p), (eye_t, eye),
                         (w3_t, w3)]:
                nc.sync.dma_start(t[:], d.ap())

            sums1 = stats_pool.tile([128, MT, NB], F32, tag="sums1")
            sq1 = stats_pool.tile([128, MT, NB], F32, tag="sq1")
            sums2 = stats_pool.tile([128, MT, NB], F32, tag="sums2")
            sq2 = stats_pool.tile([128, MT, NB], F32, tag="sq2")

            p1d = dram_pool.tile([MT, 128, BS], F32, tag="p1d")
            p2d = dram_pool.tile([MT, 128, BS], F16, tag="p2d")
            cc_in1 = dram_pool.tile([128, 2 * MT], F32, tag="cc_in1")
            cc_out1 = dram_pool.tile([128, 2 * MT], F32, tag="cc_out1")
            cc_in2 = dram_pool.tile([128, 2 * MT], F32, tag="cc_in2")
            cc_out2 = dram_pool.tile([128, 2 * MT], F32, tag="cc_out2")

            # ================= Phase 1: fc1 + prelu + stats ==================
            with (
                tc.tile_pool(name="xt", bufs=1) as xt_pool,
                tc.tile_pool(name="w1p", bufs=3) as w1_pool,
                tc.tile_pool(name="p1t", bufs=4) as p1_pool,
                tc.tile_pool(name="scr1", bufs=3) as scr_pool,
            ):
                xt_t = xt_pool.tile([128, KC1, BS], F16, tag="xt")
                for k in range(KC1):
                    nc.sync.dma_start(xt_t[:, k, :], xT.ap()[:, k, :])
                for m in range(MT):
                    w1_t = w1_pool.tile([128, KC1, 128], F16, tag="w1")
                    nc.sync.dma_start(w1_t[:], w1.ap()[m])
                    for n in range(NB):
                        ps = ps_mm.tile([128, 512], F32, tag="mm")
                        for k in range(KC1):
                            nc.tensor.matmul(
                                ps[:], w1_t[:, k, :],
                                xt_t[:, k, n * 512:(n + 1) * 512],
                                start=(k == 0), stop=(k == KC1 - 1),
                            )
                        p1_t = p1_pool.tile([128, 512], F32, tag="p1")
                        nc.scalar.activation(
                            p1_t[:], ps[:], AF.Prelu, alpha=a1_t[:],
                            scale=1.0 / FSPLIT,
                            accum_out=sums1[:, m, n:n + 1],
                        )
                        scr = scr_pool.tile([128, 512], F16, tag="scr")
                        nc.vector.scalar_tensor_tensor(
                            scr[:], p1_t[:], 0.0, p1_t[:], ALU.add, ALU.mult,
                            accum_out=sq1[:, m, n:n + 1],
                        )
                        nc.sync.dma_start(
                            p1d[m, :, n * 512:(n + 1) * 512], p1_t[:]
                        )

            # ---- BN1 stats: local reduce, AllReduce, finalize ---------------
            def bn_finalize(sums, sq, cc_in, cc_out, g_t, bt_t, tag):
                cat = stats_pool.tile([128, 2 * MT], F32, tag=f"cat{tag}")
                nc.vector.reduce_sum(cat[:, 0:MT], sums[:], axis=mybir.AxisListType.X)
                nc.vector.reduce_sum(cat[:, MT:], sq[:], axis=mybir.AxisListType.X)
                nc.sync.dma_start(cc_in[:], cat[:])
                nc.gpsimd.collective_compute(
                    "AllReduce", ALU.add,
                    replica_groups=[list(range(NCORES))],
                    ins=[cc_in.opt()], outs=[cc_out.opt()],
                )
                red = stats_pool.tile([128, 2 * MT], F32, tag=f"red{tag}")
                nc.sync.dma_start(red[:], cc_out[:])
                mu = stats_pool.tile([128, MT], F32, tag=f"mu{tag}")
                nc.vector.tensor_scalar_mul(mu[:], red[:, 0:MT], 1.0 / B)
                var = stats_pool.tile([128, MT], F32, tag=f"var{tag}")
                # var = E[p^2] - mu^2 + EPS  (fold the +EPS in here)
                nc.vector.tensor_mul(var[:], mu[:], mu[:])
                nc.vector.scalar_tensor_tensor(
                    var[:], red[:, MT:], 1.0 / B, var[:], ALU.mult, ALU.subtract,
                )
                nc.vector.tensor_scalar_add(var[:], var[:], EPS)
                rinv = stats_pool.tile([128, MT], F32, tag=f"rinv{tag}")
                nc.vector.reciprocal(rinv[:], var[:])
                r = stats_pool.tile([128, MT], F32, tag=f"r{tag}")
                nc.scalar.activation(r[:], rinv[:], AF.Sqrt)
                scale = stats_pool.tile([128, MT], F32, tag=f"scale{tag}")
                nc.vector.tensor_mul(scale[:], g_t[:], r[:])
                bias = stats_pool.tile([128, MT], F32, tag=f"bias{tag}")
                nc.vector.tensor_mul(bias[:], mu[:], scale[:])
                nc.vector.tensor_sub(bias[:], bt_t[:], bias[:])
                return cat, red, scale, bias

            cat1, red1, scale1, bias1 = bn_finalize(
                sums1, sq1, cc_in1, cc_out1, g1_t, bt1_t, "1"
            )

            # ================= Phase 2: sign + fc2 + prelu + stats ===========
            for s in range(NSUP):
                with (
                    tc.tile_pool(name=f"s1_{s}", bufs=1) as s1_pool,
                    tc.tile_pool(name=f"pin{s}", bufs=3) as pin_pool,
                    tc.tile_pool(name=f"w2_{s}", bufs=3) as w2_pool,
                    tc.tile_pool(name=f"p2_{s}", bufs=4) as p2_pool,
                    tc.tile_pool(name=f"sc2{s}", bufs=3) as scr2_pool,
                ):
                    lo = s * SUPER
                    s1_t = s1_pool.tile([128, MT, SUPER], F8, tag="s1")
                    for m in range(MT):
                        pin = pin_pool.tile([128, SUPER], F32, tag="pin")
                        nc.sync.dma_start(pin[:], p1d[m, :, lo:lo + SUPER])
                        nc.scalar.activation(
                            s1_t[:, m, :], pin[:], AF.Sign,
                            bias=bias1[:, m:m + 1], scale=scale1[:, m:m + 1],
                        )
                    for m in range(MT):
                        w2_t = w2_pool.tile([128, MT, 128], F8, tag="w2")
                        nc.sync.dma_start(w2_t[:], w2.ap()[m])
                        for j in range(NNS):
                            ps = ps_mm.tile([128, 512], F32, tag="mm")
                            for kk in range(MT // 2):
                                nc.tensor.matmul(
                                    ps[:], w2_t[:, 2 * kk:2 * kk + 2, :],
                                    s1_t[:, 2 * kk:2 * kk + 2,
                                         j * 512:(j + 1) * 512],
                                    start=(kk == 0), stop=(kk == MT // 2 - 1),
                                    perf_mode=mybir.MatmulPerfMode.DoubleRow,
                                )
                            n_g = s * NNS + j
                            p2_t = p2_pool.tile([128, 512], F16, tag="p2")
                            nc.scalar.activation(
                                p2_t[:], ps[:], AF.Prelu, alpha=a2_t[:],
                                accum_out=sums2[:, m, n_g:n_g + 1],
                            )
                            scr = scr2_pool.tile([128, 512], F16, tag="scr2")
                            nc.vector.scalar_tensor_tensor(
                                scr[:], p2_t[:], 0.0, p2_t[:], ALU.add, ALU.mult,
                                accum_out=sq2[:, m, n_g:n_g + 1],
                            )
                            nc.sync.dma_start(
                                p2d[m, :, lo + j * 512:lo + (j + 1) * 512],
                                p2_t[:],
                            )

            cat2, red2, scale2, bias2 = bn_finalize(
                sums2, sq2, cc_in2, cc_out2, g2_t, bt2_t, "2"
            )

            # ================= Phase 3: BN2 + fc3 + log_softmax ==============
            with (
                tc.tile_pool(name="qp", bufs=4) as q_pool,
                tc.tile_pool(name="lg", bufs=2) as lg_pool,
                tc.tile_pool(name="ps3", bufs=2, space="PSUM") as ps3_pool,
                tc.tile_pool(name="pst", bufs=2, space="PSUM") as pst_pool,
                tc.tile_pool(name="sm", bufs=4) as sm_pool,
                tc.tile_pool(name="op", bufs=4) as out_pool,
            ):
                for n in range(NB):
                    pl = ps3_pool.tile([C, 512], F32, tag="pl")
                    for k in range(MT):
                        qin = q_pool.tile([128, 512], F16, tag="qin")
                        nc.sync.dma_start(
                            qin[:], p2d[k, :, n * 512:(n + 1) * 512]
                        )
                        q = q_pool.tile([128, 512], F16, tag="q")
                        nc.scalar.activation(
                            q[:], qin[:], AF.Identity,
                            bias=bias2[:, k:k + 1], scale=scale2[:, k:k + 1],
                        )
                        nc.tensor.matmul(
                            pl[:], w3_t[:, k, :], q[:],
                            start=(k == 0), stop=(k == MT - 1),
                        )
                    lg = lg_pool.tile([C, 512], F32, tag="lg")
                    nc.scalar.activation(lg[:], pl[:], AF.Identity, bias=b3_t[:])
                    for j in range(4):
                        pt = pst_pool.tile([128, C], F32, tag="pt")
                        nc.tensor.transpose(
                            pt[:], lg[:, j * 128:(j + 1) * 128], eye_t[:]
                        )
                        mx = sm_pool.tile([128, 1], F32, tag="mx")
                        nc.vector.reduce_max(
                            mx[:], pt[:], axis=mybir.AxisListType.X, negate=True
                        )
                        ex = sm_pool.tile([128, C], F32, tag="ex")
                        se = sm_pool.tile([128, 1], F32, tag="se")
                        nc.scalar.activation(
                            ex[:], pt[:], AF.Exp, bias=mx[:], accum_out=se[:]
                        )
                        ln = sm_pool.tile([128, 1], F32, tag="ln")
                        nc.scalar.activation(ln[:], se[:], AF.Ln)
                        adj = sm_pool.tile([128, 1], F32, tag="adj")
                        nc.vector.tensor_sub(adj[:], mx[:], ln[:])
                        ot = out_pool.tile([128, C], F32, tag="ot")
                        nc.vector.tensor_scalar(
                            ot[:], pt[:], adj[:], None, ALU.add
                        )
                        nc.sync.dma_start(
                            out.ap()[n * 512 + j * 128:n * 512 + (j + 1) * 128, :],
                            ot[:],
                        )

            if debug:
                for nm, t in [("dbg_stats1", cat1), ("dbg_red1", red1),
                              ("dbg_scale1", scale1), ("dbg_bias1", bias1),
                              ("dbg_red2", red2)]:
                    nc.sync.dma_start(dbg[nm].ap(), t[:])
                d1f = stats_pool.tile([128, 512], F32, tag="d1f")
                nc.sync.dma_start(d1f[:], p1d[0, :, 0:512])
                nc.sync.dma_start(dbg["dbg_p1"].ap(), d1f[:])
                d2 = stats_pool.tile([128, 512], F16, tag="d2")
                nc.sync.dma_start(d2[:], p2d[0, :, 0:512])
                d2f = stats_pool.tile([128, 512], F32, tag="d2f")
                nc.vector.tensor_copy(d2f[:], d2[:])
                nc.sync.dma_start(dbg["dbg_p2"].ap(), d2f[:])

    nc.compile()
    return nc


def prep_inputs(x, W1, b1, a1, g1, beta1, W2, a2, g2, beta2, W3, b3):
    """Host-side layout prep. Returns per-core in_maps."""
    x = np.ascontiguousarray(np.asarray(x, np.float32))
    W1 = np.asarray(W1, np.float32)
    b1 = np.asarray(b1, np.float32)
    W2 = np.asarray(W2, np.float32)
    W3 = np.asarray(W3, np.float32)
    b3 = np.asarray(b3, np.float32)

    # fc1 operands with bias folded in as contraction row 784 (rows 785+ zero).
    # fp16 hi/lo split with 2^11 scaling, packed along K:
    #   XF = [xh; xh; xl*S],  WF = [wh*S; wl*S; wh]  ->  psum = S * h1
    # where v = vh + vl exactly captures ~22 mantissa bits.  The bias row uses
    # x-side 32.0 / w-side b1/32 to keep w*S within fp16 range.
    S = np.float32(FSPLIT)
    xT_aug = np.zeros((D + 1, B), np.float32)
    xT_aug[0:D] = x.T
    xT_aug[D] = 32.0
    w1T_aug = np.zeros((D + 1, H1), np.float32)
    w1T_aug[0:D] = W1.T
    w1T_aug[D] = b1 / 32.0

    xh = xT_aug.astype(np.float16)
    xl = ((xT_aug - xh.astype(np.float32)) * S).astype(np.float16)
    wh = w1T_aug.astype(np.float16)
    whs = (w1T_aug * S).astype(np.float16)
    wls = ((w1T_aug - wh.astype(np.float32)) * S).astype(np.float16)
    KPAD = KC1 * 128
    A = D + 1
    xF = np.zeros((KPAD, B), np.float16)
    xF[0:A] = xh
    xF[A:2 * A] = xh
    xF[2 * A:2 * A + D] = xl[0:D]
    wF = np.zeros((KPAD, H1), np.float16)
    wF[0:A] = whs
    wF[A:2 * A] = wls
    wF[2 * A:2 * A + D] = wh[0:D]
    w1_blk = np.ascontiguousarray(
        wF.reshape(KC1, 128, MT, 128).transpose(2, 1, 0, 3)
    )

    sW2T = np.where(W2 >= 0, np.float32(1), np.float32(-1)).T
    w2_blk = np.ascontiguousarray(
        sW2T.reshape(MT, 128, MT, 128).transpose(2, 1, 0, 3)
    ).astype(ml_dtypes.float8_e4m3)

    w3_blk = np.ascontiguousarray(
        W3.T.reshape(MT, 128, C).transpose(1, 0, 2)
    ).astype(np.float16)

    def feat_layout(v):
        return np.ascontiguousarray(np.asarray(v, np.float32).reshape(MT, 128).T)

    shared = dict(
        w1=w1_blk, w2=w2_blk, w3=w3_blk,
        g1=feat_layout(g1), bt1=feat_layout(beta1),
        g2=feat_layout(g2), bt2=feat_layout(beta2),
        a1p=np.full((128, 1), np.float32(a1), np.float32),
        a2p=np.full((128, 1), np.float32(a2), np.float32),
        b3p=b3.reshape(C, 1).astype(np.float32),
        eye=np.eye(C, dtype=np.float32),
    )
    in_maps = []
    for c in range(NCORES):
        sl = xF[:, c * BS:(c + 1) * BS]
        xs = np.ascontiguousarray(sl.reshape(KC1, 128, BS).transpose(1, 0, 2))
        in_maps.append(dict(shared, xT=xs))
    return in_maps


_NC_CACHE = {}


def run(inputs, debug=False, trace=False):
    key = (debug,)
    if key not in _NC_CACHE:
        _NC_CACHE[key] = build_program(debug=debug)
    nc = _NC_CACHE[key]
    in_maps = prep_inputs(**inputs)
    res = run_bass_kernel_spmd(
        nc, in_maps, core_ids=list(range(NCORES)), trace=trace
    )
    outs = np.concatenate([res.results[c]["out"] for c in range(NCORES)], axis=0)
    return outs, res


def kernel(**inputs):
    out, _ = run(inputs)
    return out


# revision 29
# speedup vs baseline: 1.5383x; 1.0354x over previous
"""BinaryMLP (nn_BinaryMLP_91276644974884) on 8 TRN2 NeuronCores.

Reference network (B=32768, D=784, H1=H2=4096, C=10):
    h  = x @ W1.T + b1                    # fc1
    h  = BN1(prelu(h, a1)) (batch stats)
    h  = sign(h) @ sign(W2).T             # fc2, binary GEMM
    h  = BN2(prelu(h, a2))
    o  = log_softmax(h @ W3.T + b3)

Strategy: data-parallel over batch (4096 rows/core), everything computed in a
transposed [features, batch] layout so BatchNorm stats are free-axis
reductions.  fc1 runs in float32r (measured err std ~1.5e-4 — sign-safe);
fc2 in bf16 which is EXACT for +-1 inputs (fp32 PSUM accumulate); the two
BatchNorm full-batch statistics are tiny [128, 64] AllReduces across the 8
cores.  prelu+row-sums fuse into one ScalarE Prelu pass; sign+BN fuse into
one ScalarE Sign pass (out = Sign(scale*p + bias)).

Host-side prep (free - not on device critical path): transposes/blocked
weight layouts, sign(W2) cast to bf16, bias rows folded into fc1's
contraction dim (row 784 of x^T is ones, of W1^T is b1).
"""

import numpy as np
import ml_dtypes

import concourse.bass as bass
import concourse.tile as tile
from concourse import bacc, mybir
from concourse.bass_utils import run_bass_kernel_spmd

F32 = mybir.dt.float32
F32R = mybir.dt.float32r
F16 = mybir.dt.float16
BF16 = mybir.dt.bfloat16
F8 = mybir.dt.float8e4
AF = mybir.ActivationFunctionType
ALU = mybir.AluOpType

NCORES = 8
B = 32768
BS = B // NCORES          # 4096 batch rows per core
D = 784
K1ROWS = 2 * (D + 1) + D  # 2354: [xh+bias; xh+bias; xl] tightly packed along K
KC1 = -(-K1ROWS // 128)   # 19 chunks (padded to 2432)
FSPLIT = 2048.0           # 2^11 hi/lo split scale
H1 = 4096
H2 = 4096
MT = 32                   # 4096 / 128 feature tiles
C = 10
SUPER = 2048              # batch super-chunk resident in SBUF during fc2
NSUP = BS // SUPER        # 2
NNS = SUPER // 512        # 4 512-col chunks per super
NB = BS // 512            # 8 512-col chunks per core
EPS = 1e-5
NG = 4                    # BN stat groups per phase (pipelined AllReduces)
GM = MT // NG             # 8 feature tiles per group


def build_program(debug=False):
    nc = bacc.Bacc("TRN2", target_bir_lowering=False, debug=False,
                   num_devices=NCORES)

    xT = nc.declare_dram_parameter("xT", [128, KC1, BS], F16, isOutput=False)
    w1 = nc.declare_dram_parameter("w1", [MT, 128, KC1, 128], F16, isOutput=False)
    w2 = nc.declare_dram_parameter("w2", [MT, 128, MT, 128], F8, isOutput=False)
    w3 = nc.declare_dram_parameter("w3", [128, MT, C], F16, isOutput=False)
    g1 = nc.declare_dram_parameter("g1", [128, MT], F32, isOutput=False)
    bt1 = nc.declare_dram_parameter("bt1", [128, MT], F32, isOutput=False)
    g2 = nc.declare_dram_parameter("g2", [128, MT], F32, isOutput=False)
    bt2 = nc.declare_dram_parameter("bt2", [128, MT], F32, isOutput=False)
    a1p = nc.declare_dram_parameter("a1p", [128, 1], F32, isOutput=False)
    a2p = nc.declare_dram_parameter("a2p", [128, 1], F32, isOutput=False)
    b3p = nc.declare_dram_parameter("b3p", [C, 1], F32, isOutput=False)
    eye = nc.declare_dram_parameter("eye", [C, C], F32, isOutput=False)
    out = nc.declare_dram_parameter("out", [BS, C], F32, isOutput=True)

    dbg = {}
    if debug:
        for nm, shp in [
            ("dbg_stats1", [128, 2 * GM]), ("dbg_red1", [128, 2 * GM]),
            ("dbg_scale1", [128, MT]), ("dbg_bias1", [128, MT]),
            ("dbg_red2", [128, 2 * GM]),
            ("dbg_p1", [128, 512]), ("dbg_p2", [128, 512]),
        ]:
            dbg[nm] = nc.declare_dram_parameter(nm, shp, F32, isOutput=True)

    with tile.TileContext(nc) as tc:
        with (
            tc.tile_pool(name="const", bufs=1) as const_pool,
            tc.tile_pool(name="stats", bufs=1) as stats_pool,
            tc.tile_pool(name="dram", bufs=1, space="DRAM") as dram_pool,
            tc.tile_pool(name="psmm", bufs=4, space="PSUM") as ps_mm,
        ):
            # ---- persistent small tiles -------------------------------------
            g1_t = const_pool.tile([128, MT], F32, tag="g1")
            bt1_t = const_pool.tile([128, MT], F32, tag="bt1")
            g2_t = const_pool.tile([128, MT], F32, tag="g2")
            bt2_t = const_pool.tile([128, MT], F32, tag="bt2")
            a1_t = const_pool.tile([128, 1], F32, tag="a1")
            a2_t = const_pool.tile([128, 1], F32, tag="a2")
            b3_t = const_pool.tile([C, 1], F32, tag="b3")
            eye_t = const_pool.tile([C, C], F32, tag="eye")
            w3_t = const_pool.tile([128, MT, C], F16, tag="w3")
            for t, d in [(g1_t, g1), (bt1_t, bt1), (g2_t, g2), (bt2_t, bt2),
                         (a1_t, a1p), (a2_t, a2p), (b3_t, b3p), (eye_t, eye),
                         (w3_t, w3)]:
                nc.sync.dma_start(t[:], d.ap())

            sums1 = stats_pool.tile([128, MT, NB], F32, tag="sums1")
            sq1 = stats_pool.tile([128, MT, NB], F32, tag="sq1")
            sums2 = stats_pool.tile([128, MT, NB], F32, tag="sums2")
            sq2 = stats_pool.tile([128, MT, NB], F32, tag="sq2")

            p1d = dram_pool.tile([MT, 128, BS], F32, tag="p1d")
            p2d = dram_pool.tile([MT, 128, BS], F16, tag="p2d")
            cc_in1 = dram_pool.tile([NG, 128, 2 * GM], F32, tag="cc_in1")
            cc_out1 = dram_pool.tile([NG, 128, 2 * GM], F32, tag="cc_out1")
            cc_in2 = dram_pool.tile([NG, 128, 2 * GM], F32, tag="cc_in2")
            cc_out2 = dram_pool.tile([NG, 128, 2 * GM], F32, tag="cc_out2")

            # ---- BN stats: per-group local reduce, AllReduce, finalize.
            # Emitted inside the fc1/fc2 loops as each group's 8 feature
            # tiles complete, so collectives + Sign/affine prep overlap the
            # remaining matmul stream.
            scale1 = stats_pool.tile([128, MT], F32, tag="scale1")
            bias1 = stats_pool.tile([128, MT], F32, tag="bias1")
            scale2 = stats_pool.tile([128, MT], F32, tag="scale2")
            bias2 = stats_pool.tile([128, MT], F32, tag="bias2")
            cats = {}

            def bn_group(sums, sq, cc_in, cc_out, g_t, bt_t, scale, bias,
                         g, tag):
                """Finalize BN scale/bias for feature tiles g*GM..(g+1)*GM-1."""
                msl = slice(g * GM, (g + 1) * GM)
                cat = stats_pool.tile([128, 2 * GM], F32, tag=f"cat{tag}_{g}",
                                      name=f"cat{tag}_{g}")
                nc.vector.reduce_sum(cat[:, 0:GM], sums[:, msl, :],
                                     axis=mybir.AxisListType.X)
                nc.vector.reduce_sum(cat[:, GM:], sq[:, msl, :],
                                     axis=mybir.AxisListType.X)
                nc.sync.dma_start(cc_in[g], cat[:])
                nc.gpsimd.collective_compute(
                    "AllReduce", ALU.add,
                    replica_groups=[list(range(NCORES))],
                    ins=[cc_in[g].opt()], outs=[cc_out[g].opt()],
                )
                red = stats_pool.tile([128, 2 * GM], F32, tag=f"red{tag}_{g}",
                                      name=f"red{tag}_{g}")
                nc.sync.dma_start(red[:], cc_out[g])
                cats[f"cat{tag}_{g}"] = cat
                cats[f"red{tag}_{g}"] = red
                mu = stats_pool.tile([128, GM], F32, tag=f"mu{tag}_{g}",
                                     name=f"mu{tag}_{g}")
                nc.vector.tensor_scalar_mul(mu[:], red[:, 0:GM], 1.0 / B)
                var = stats_pool.tile([128, GM], F32, tag=f"var{tag}_{g}",
                                      name=f"var{tag}_{g}")
                # var = E[p^2] - mu^2 + EPS  (fold the +EPS in here)
                nc.vector.tensor_mul(var[:], mu[:], mu[:])
                nc.vector.scalar_tensor_tensor(
                    var[:], red[:, GM:], 1.0 / B, var[:], ALU.mult, ALU.subtract,
                )
                nc.vector.tensor_scalar_add(var[:], var[:], EPS)
                rinv = stats_pool.tile([128, GM], F32, tag=f"rinv{tag}_{g}",
                                       name=f"rinv{tag}_{g}")
                nc.vector.reciprocal(rinv[:], var[:])
                r = stats_pool.tile([128, GM], F32, tag=f"r{tag}_{g}",
                                    name=f"r{tag}_{g}")
                nc.scalar.activation(r[:], rinv[:], AF.Sqrt)
                nc.vector.tensor_mul(scale[:, msl], g_t[:, msl], r[:])
                nc.vector.tensor_mul(bias[:, msl], mu[:], scale[:, msl])
                nc.vector.tensor_sub(bias[:, msl], bt_t[:, msl], bias[:, msl])

            # ================= Phase 1: fc1 + prelu + stats ==================
            with (
                tc.tile_pool(name="xt", bufs=1) as xt_pool,
                tc.tile_pool(name="w1p", bufs=4) as w1_pool,
                tc.tile_pool(name="p1t", bufs=4) as p1_pool,
                tc.tile_pool(name="scr1", bufs=3) as scr_pool,
            ):
                xt_t = xt_pool.tile([128, KC1, BS], F16, tag="xt")
                for n in range(NB):
                    nc.sync.dma_start(
                        xt_t[:, :, n * 512:(n + 1) * 512],
                        xT.ap()[:, :, n * 512:(n + 1) * 512],
                    )
                for m in range(MT):
                    w1_t = w1_pool.tile([128, KC1, 128], F16, tag="w1")
                    nc.sync.dma_start(w1_t[:], w1.ap()[m])
                    for n in range(NB):
                        ps = ps_mm.tile([128, 512], F32, tag="mm")
                        for k in range(KC1):
                            nc.tensor.matmul(
                                ps[:], w1_t[:, k, :],
                                xt_t[:, k, n * 512:(n + 1) * 512],
                                start=(k == 0), stop=(k == KC1 - 1),
                            )
                        p1_t = p1_pool.tile([128, 512], F32, tag="p1")
                        nc.scalar.activation(
                            p1_t[:], ps[:], AF.Prelu, alpha=a1_t[:],
                            scale=1.0 / FSPLIT,
                            accum_out=sums1[:, m, n:n + 1],
                        )
                        scr = scr_pool.tile([128, 512], F16, tag="scr")
                        nc.vector.scalar_tensor_tensor(
                            scr[:], p1_t[:], 0.0, p1_t[:], ALU.add, ALU.mult,
                            accum_out=sq1[:, m, n:n + 1],
                        )
                        nc.sync.dma_start(
                            p1d[m, :, n * 512:(n + 1) * 512], p1_t[:]
                        )
                    if m % GM == GM - 1:
                        bn_group(sums1, sq1, cc_in1, cc_out1, g1_t, bt1_t,
                                 scale1, bias1, m // GM, "1")

            # ================= Phase 2: sign + fc2 + prelu + stats ===========
            for s in range(NSUP):
                with (
                    tc.tile_pool(name=f"s1_{s}", bufs=1) as s1_pool,
                    tc.tile_pool(name=f"pin{s}", bufs=4) as pin_pool,
                    tc.tile_pool(name=f"w2_{s}", bufs=4) as w2_pool,
                    tc.tile_pool(name=f"p2_{s}", bufs=4) as p2_pool,
                    tc.tile_pool(name=f"sc2{s}", bufs=3) as scr2_pool,
                ):
                    lo = s * SUPER
                    s1_t = s1_pool.tile([128, MT, SUPER], F8, tag="s1")
                    for m in range(MT):
                        pin = pin_pool.tile([128, SUPER], F32, tag="pin")
                        nc.sync.dma_start(pin[:], p1d[m, :, lo:lo + SUPER])
                        nc.scalar.activation(
                            s1_t[:, m, :], pin[:], AF.Sign,
                            bias=bias1[:, m:m + 1], scale=scale1[:, m:m + 1],
                        )
                    for m in range(MT):
                        w2_t = w2_pool.tile([128, MT, 128], F8, tag="w2")
                        nc.sync.dma_start(w2_t[:], w2.ap()[m])
                        for j in range(NNS):
                            ps = ps_mm.tile([128, 512], F32, tag="mm")
                            for kk in range(MT // 2):
                                nc.tensor.matmul(
                                    ps[:], w2_t[:, 2 * kk:2 * kk + 2, :],
                                    s1_t[:, 2 * kk:2 * kk + 2,
                                         j * 512:(j + 1) * 512],
                                    start=(kk == 0), stop=(kk == MT // 2 - 1),
                                    perf_mode=mybir.MatmulPerfMode.DoubleRow,
                                )
                            n_g = s * NNS + j
                            p2_t = p2_pool.tile([128, 512], F16, tag="p2")
                            nc.scalar.activation(
                                p2_t[:], ps[:], AF.Prelu, alpha=a2_t[:],
                                accum_out=sums2[:, m, n_g:n_g + 1],
                            )
                            scr = scr2_pool.tile([128, 512], F16, tag="scr2")
                            nc.vector.scalar_tensor_tensor(
                                scr[:], p2_t[:], 0.0, p2_t[:], ALU.add, ALU.mult,
                                accum_out=sq2[:, m, n_g:n_g + 1],
                            )
                            nc.sync.dma_start(
                                p2d[m, :, lo + j * 512:lo + (j + 1) * 512],
                                p2_t[:],
                            )
                        if s == NSUP - 1 and m % GM == GM - 1:
                            bn_group(sums2, sq2, cc_in2, cc_out2, g2_t, bt2_t,
                                     scale2, bias2, m // GM, "2")

            # ================= Phase 3: BN2 + fc3 + log_softmax ==============
            with (
                tc.tile_pool(name="qp", bufs=8) as q_pool,
                tc.tile_pool(name="lg", bufs=2) as lg_pool,
                tc.tile_pool(name="ps3", bufs=2, space="PSUM") as ps3_pool,
                tc.tile_pool(name="pst", bufs=2, space="PSUM") as pst_pool,
                tc.tile_pool(name="sm", bufs=4) as sm_pool,
                tc.tile_pool(name="op", bufs=4) as out_pool,
            ):
                for n in range(NB):
                    pl = ps3_pool.tile([C, 512], F32, tag="pl")
                    for k in range(MT):
                        qin = q_pool.tile([128, 512], F16, tag="qin")
                        nc.sync.dma_start(
                            qin[:], p2d[k, :, n * 512:(n + 1) * 512]
                        )
                        q = q_pool.tile([128, 512], F16, tag="q")
                        nc.vector.tensor_scalar(
                            q[:], qin[:], scale2[:, k:k + 1],
                            bias2[:, k:k + 1], ALU.mult, ALU.add,
                        )
                        nc.tensor.matmul(
                            pl[:], w3_t[:, k, :], q[:],
                            start=(k == 0), stop=(k == MT - 1),
                        )
                    lg = lg_pool.tile([C, 512], F32, tag="lg")
                    nc.scalar.activation(lg[:], pl[:], AF.Identity, bias=b3_t[:])
                    for j in range(4):
                        pt = pst_pool.tile([128, C], F32, tag="pt")
                        nc.tensor.transpose(
                            pt[:], lg[:, j * 128:(j + 1) * 128], eye_t[:]
                        )
                        mx = sm_pool.tile([128, 1], F32, tag="mx")
                        nc.vector.reduce_max(
                            mx[:], pt[:], axis=mybir.AxisListType.X, negate=True
                        )
                        ex = sm_pool.tile([128, C], F32, tag="ex")
                        se = sm_pool.tile([128, 1], F32, tag="se")
                        nc.scalar.activation(
                            ex[:], pt[:], AF.Exp, bias=mx[:], accum_out=se[:]
                        )
                        ln = sm_pool.tile([128, 1], F32, tag="ln")
                        nc.scalar.activation(ln[:], se[:], AF.Ln)
                        adj = sm_pool.tile([128, 1], F32, tag="adj")
                        nc.vector.tensor_sub(adj[:], mx[:], ln[:])
                        ot = out_pool.tile([128, C], F32, tag="ot")
                        nc.vector.tensor_scalar(
                            ot[:], pt[:], adj[:], None, ALU.add
                        )
                        nc.sync.dma_start(
                            out.ap()[n * 512 + j * 128:n * 512 + (j + 1) * 128, :],
                            ot[:],
                        )

            if debug:
                for nm, t in [("dbg_stats1", cats["cat1_0"]),
                              ("dbg_red1", cats["red1_0"]),
                              ("dbg_scale1", scale1), ("dbg_bias1", bias1),
                              ("dbg_red2", cats["red2_0"])]:
                    nc.sync.dma_start(dbg[nm].ap(), t[:])
                d1f = stats_pool.tile([128, 512], F32, tag="d1f")
                nc.sync.dma_start(d1f[:], p1d[0, :, 0:512])
                nc.sync.dma_start(dbg["dbg_p1"].ap(), d1f[:])
                d2 = stats_pool.tile([128, 512], F16, tag="d2")
                nc.sync.dma_start(d2[:], p2d[0, :, 0:512])
                d2f = stats_pool.tile([128, 512], F32, tag="d2f")
                nc.vector.tensor_copy(d2f[:], d2[:])
                nc.sync.dma_start(dbg["dbg_p2"].ap(), d2f[:])

    nc.compile()
    return nc


def prep_inputs(x, W1, b1, a1, g1, beta1, W2, a2, g2, beta2, W3, b3):
    """Host-side layout prep. Returns per-core in_maps."""
    x = np.ascontiguousarray(np.asarray(x, np.float32))
    W1 = np.asarray(W1, np.float32)
    b1 = np.asarray(b1, np.float32)
    W2 = np.asarray(W2, np.float32)
    W3 = np.asarray(W3, np.float32)
    b3 = np.asarray(b3, np.float32)

    # fc1 operands with bias folded in as contraction row 784 (rows 785+ zero).
    # fp16 hi/lo split with 2^11 scaling, packed along K:
    #   XF = [xh; xh; xl*S],  WF = [wh*S; wl*S; wh]  ->  psum = S * h1
    # where v = vh + vl exactly captures ~22 mantissa bits.  The bias row uses
    # x-side 32.0 / w-side b1/32 to keep w*S within fp16 range.
    S = np.float32(FSPLIT)
    xT_aug = np.zeros((D + 1, B), np.float32)
    xT_aug[0:D] = x.T
    xT_aug[D] = 32.0
    w1T_aug = np.zeros((D + 1, H1), np.float32)
    w1T_aug[0:D] = W1.T
    w1T_aug[D] = b1 / 32.0

    xh = xT_aug.astype(np.float16)
    xl = ((xT_aug - xh.astype(np.float32)) * S).astype(np.float16)
    wh = w1T_aug.astype(np.float16)
    whs = (w1T_aug * S).astype(np.float16)
    wls = ((w1T_aug - wh.astype(np.float32)) * S).astype(np.float16)
    KPAD = KC1 * 128
    A = D + 1
    xF = np.zeros((KPAD, B), np.float16)
    xF[0:A] = xh
    xF[A:2 * A] = xh
    xF[2 * A:2 * A + D] = xl[0:D]
    wF = np.zeros((KPAD, H1), np.float16)
    wF[0:A] = whs
    wF[A:2 * A] = wls
    wF[2 * A:2 * A + D] = wh[0:D]
    w1_blk = np.ascontiguousarray(
        wF.reshape(KC1, 128, MT, 128).transpose(2, 1, 0, 3)
    )

    sW2T = np.where(W2 >= 0, np.float32(1), np.float32(-1)).T
    w2_blk = np.ascontiguousarray(
        sW2T.reshape(MT, 128, MT, 128).transpose(2, 1, 0, 3)
    ).astype(ml_dtypes.float8_e4m3)

    w3_blk = np.ascontiguousarray(
        W3.T.reshape(MT, 128, C).transpose(1, 0, 2)
    ).astype(np.float16)

    def feat_layout(v):
        return np.ascontiguousarray(np.asarray(v, np.float32).reshape(MT, 128).T)

    shared = dict(
        w1=w1_blk, w2=w2_blk, w3=w3_blk,
        g1=feat_layout(g1), bt1=feat_layout(beta1),
        g2=feat_layout(g2), bt2=feat_layout(beta2),
        a1p=np.full((128, 1), np.float32(a1), np.float32),
        a2p=np.full((128, 1), np.float32(a2), np.float32),
        b3p=b3.reshape(C, 1).astype(np.float32),
        eye=np.eye(C, dtype=np.float32),
    )
    in_maps = []
    for c in range(NCORES):
        sl = xF[:, c * BS:(c + 1) * BS]
        xs = np.ascontiguousarray(sl.reshape(KC1, 128, BS).transpose(1, 0, 2))
        in_maps.append(dict(shared, xT=xs))
    return in_maps


_NC_CACHE = {}


def run(inputs, debug=False, trace=False):
    key = (debug,)
    if key not in _NC_CACHE:
        _NC_CACHE[key] = build_program(debug=debug)
    nc = _NC_CACHE[key]
    in_maps = prep_inputs(**inputs)
    res = run_bass_kernel_spmd(
        nc, in_maps, core_ids=list(range(NCORES)), trace=trace
    )
    outs = np.concatenate([res.results[c]["out"] for c in range(NCORES)], axis=0)
    return outs, res


def kernel(**inputs):
    out, _ = run(inputs)
    return out


# revision 35
# speedup vs baseline: 1.5452x; 1.0045x over previous
"""BinaryMLP (nn_BinaryMLP_91276644974884) on 8 TRN2 NeuronCores.

Reference network (B=32768, D=784, H1=H2=4096, C=10):
    h  = x @ W1.T + b1                    # fc1
    h  = BN1(prelu(h, a1)) (batch stats)
    h  = sign(h) @ sign(W2).T             # fc2, binary GEMM
    h  = BN2(prelu(h, a2))
    o  = log_softmax(h @ W3.T + b3)

Strategy: data-parallel over batch (4096 rows/core), everything computed in a
transposed [features, batch] layout so BatchNorm stats are free-axis
reductions.  fc1 runs in float32r (measured err std ~1.5e-4 — sign-safe);
fc2 in bf16 which is EXACT for +-1 inputs (fp32 PSUM accumulate); the two
BatchNorm full-batch statistics are tiny [128, 64] AllReduces across the 8
cores.  prelu+row-sums fuse into one ScalarE Prelu pass; sign+BN fuse into
one ScalarE Sign pass (out = Sign(scale*p + bias)).

Host-side prep (free - not on device critical path): transposes/blocked
weight layouts, sign(W2) cast to bf16, bias rows folded into fc1's
contraction dim (row 784 of x^T is ones, of W1^T is b1).
"""

import numpy as np
import ml_dtypes

import concourse.bass as bass
import concourse.tile as tile
from concourse import bacc, mybir
from concourse.bass_utils import run_bass_kernel_spmd

F32 = mybir.dt.float32
F32R = mybir.dt.float32r
F16 = mybir.dt.float16
BF16 = mybir.dt.bfloat16
F8 = mybir.dt.float8e4
AF = mybir.ActivationFunctionType
ALU = mybir.AluOpType

NCORES = 8
B = 32768
BS = B // NCORES          # 4096 batch rows per core
D = 784
K1ROWS = 2 * (D + 1) + D  # 2354: [xh+bias; xh+bias; xl] tightly packed along K
KC1 = -(-K1ROWS // 128)   # 19 chunks (padded to 2432)
FSPLIT = 2048.0           # 2^11 hi/lo split scale
H1 = 4096
H2 = 4096
MT = 32                   # 4096 / 128 feature tiles
C = 10
SUPER = 2048              # batch super-chunk resident in SBUF during fc2
NSUP = BS // SUPER        # 2
NNS = SUPER // 512        # 4 512-col chunks per super
NB = BS // 512            # 8 512-col chunks per core
EPS = 1e-5
NG = 4                    # BN stat groups per phase (pipelined AllReduces)
GM = MT // NG             # 8 feature tiles per group


def build_program(debug=False):
    nc = bacc.Bacc("TRN2", target_bir_lowering=False, debug=False,
                   num_devices=NCORES)

    xT = nc.declare_dram_parameter("xT", [128, NB, KC1, 512], F16,
                                   isOutput=False)
    w1 = nc.declare_dram_parameter("w1", [MT, 128, KC1, 128], F16, isOutput=False)
    w2 = nc.declare_dram_parameter("w2", [MT, 128, MT, 128], F8, isOutput=False)
    w3 = nc.declare_dram_parameter("w3", [128, MT, C], F16, isOutput=False)
    g1 = nc.declare_dram_parameter("g1", [128, MT], F32, isOutput=False)
    bt1 = nc.declare_dram_parameter("bt1", [128, MT], F32, isOutput=False)
    g2 = nc.declare_dram_parameter("g2", [128, MT], F32, isOutput=False)
    bt2 = nc.declare_dram_parameter("bt2", [128, MT], F32, isOutput=False)
    a1p = nc.declare_dram_parameter("a1p", [128, 1], F32, isOutput=False)
    a2p = nc.declare_dram_parameter("a2p", [128, 1], F32, isOutput=False)
    b3p = nc.declare_dram_parameter("b3p", [C, 1], F32, isOutput=False)
    eye = nc.declare_dram_parameter("eye", [C, C], F32, isOutput=False)
    out = nc.declare_dram_parameter("out", [BS, C], F32, isOutput=True)

    dbg = {}
    if debug:
        for nm, shp in [
            ("dbg_stats1", [128, 2 * GM]), ("dbg_red1", [128, 2 * GM]),
            ("dbg_scale1", [128, MT]), ("dbg_bias1", [128, MT]),
            ("dbg_red2", [128, 2 * GM]),
            ("dbg_p1", [128, 512]), ("dbg_p2", [128, 512]),
        ]:
            dbg[nm] = nc.declare_dram_parameter(nm, shp, F32, isOutput=True)

    with tile.TileContext(nc) as tc:
        with (
            tc.tile_pool(name="const", bufs=1) as const_pool,
            tc.tile_pool(name="stats", bufs=1) as stats_pool,
            tc.tile_pool(name="dram", bufs=1, space="DRAM") as dram_pool,
            tc.tile_pool(name="psmm", bufs=4, space="PSUM") as ps_mm,
            tc.tile_pool(name="pin", bufs=2) as pin_pool,
            tc.tile_pool(name="s1s", bufs=2) as s1s_pool,
        ):
            # ---- persistent small tiles -------------------------------------
            g1_t = const_pool.tile([128, MT], F32, tag="g1")
            bt1_t = const_pool.tile([128, MT], F32, tag="bt1")
            g2_t = const_pool.tile([128, MT], F32, tag="g2")
            bt2_t = const_pool.tile([128, MT], F32, tag="bt2")
            a1_t = const_pool.tile([128, 1], F32, tag="a1")
            a2_t = const_pool.tile([128, 1], F32, tag="a2")
            b3_t = const_pool.tile([C, 1], F32, tag="b3")
            eye_t = const_pool.tile([C, C], F32, tag="eye")
            w3_t = const_pool.tile([128, MT, C], F16, tag="w3")
            for t, d in [(g1_t, g1), (bt1_t, bt1), (g2_t, g2), (bt2_t, bt2),
                         (a1_t, a1p), (a2_t, a2p), (b3_t, b3p), (eye_t, eye),
                         (w3_t, w3)]:
                nc.sync.dma_start(t[:], d.ap())

            sums1 = stats_pool.tile([128, MT, NB], F32, tag="sums1")
            sq1 = stats_pool.tile([128, MT, NB], F32, tag="sq1")
            sums2 = stats_pool.tile([128, MT, NB], F32, tag="sums2")
            sq2 = stats_pool.tile([128, MT, NB], F32, tag="sq2")

            p1d = dram_pool.tile([MT, 128, BS], F32, tag="p1d")
            p2d = dram_pool.tile([MT, 128, BS], F16, tag="p2d")
            s1d = dram_pool.tile([MT, 128, BS], F8, tag="s1d")
            cc_in1 = dram_pool.tile([NG, 128, 2 * GM], F32, tag="cc_in1")
            cc_out1 = dram_pool.tile([NG, 128, 2 * GM], F32, tag="cc_out1")
            cc_in2 = dram_pool.tile([NG, 128, 2 * GM], F32, tag="cc_in2")
            cc_out2 = dram_pool.tile([NG, 128, 2 * GM], F32, tag="cc_out2")

            # ---- BN stats: per-group local reduce, AllReduce, finalize.
            # Emitted inside the fc1/fc2 loops as each group's 8 feature
            # tiles complete, so collectives + Sign/affine prep overlap the
            # remaining matmul stream.
            scale1 = stats_pool.tile([128, MT], F32, tag="scale1")
            bias1 = stats_pool.tile([128, MT], F32, tag="bias1")
            scale2 = stats_pool.tile([128, MT], F32, tag="scale2")
            bias2 = stats_pool.tile([128, MT], F32, tag="bias2")
            cats = {}

            def bn_group(sums, sq, cc_in, cc_out, g_t, bt_t, scale, bias,
                         g, tag):
                """Finalize BN scale/bias for feature tiles g*GM..(g+1)*GM-1."""
                msl = slice(g * GM, (g + 1) * GM)
                cat = stats_pool.tile([128, 2 * GM], F32, tag=f"cat{tag}_{g}",
                                      name=f"cat{tag}_{g}")
                nc.vector.reduce_sum(cat[:, 0:GM], sums[:, msl, :],
                                     axis=mybir.AxisListType.X)
                nc.vector.reduce_sum(cat[:, GM:], sq[:, msl, :],
                                     axis=mybir.AxisListType.X)
                nc.sync.dma_start(cc_in[g], cat[:])
                nc.gpsimd.collective_compute(
                    "AllReduce", ALU.add,
                    replica_groups=[list(range(NCORES))],
                    ins=[cc_in[g].opt()], outs=[cc_out[g].opt()],
                )
                red = stats_pool.tile([128, 2 * GM], F32, tag=f"red{tag}_{g}",
                                      name=f"red{tag}_{g}")
                nc.sync.dma_start(red[:], cc_out[g])
                cats[f"cat{tag}_{g}"] = cat
                cats[f"red{tag}_{g}"] = red
                mu = stats_pool.tile([128, GM], F32, tag=f"mu{tag}_{g}",
                                     name=f"mu{tag}_{g}")
                nc.vector.tensor_scalar_mul(mu[:], red[:, 0:GM], 1.0 / B)
                var = stats_pool.tile([128, GM], F32, tag=f"var{tag}_{g}",
                                      name=f"var{tag}_{g}")
                # var = E[p^2] - mu^2 + EPS  (fold the +EPS in here)
                nc.vector.tensor_mul(var[:], mu[:], mu[:])
                nc.vector.scalar_tensor_tensor(
                    var[:], red[:, GM:], 1.0 / B, var[:], ALU.mult, ALU.subtract,
                )
                nc.vector.tensor_scalar_add(var[:], var[:], EPS)
                rinv = stats_pool.tile([128, GM], F32, tag=f"rinv{tag}_{g}",
                                       name=f"rinv{tag}_{g}")
                nc.vector.reciprocal(rinv[:], var[:])
                r = stats_pool.tile([128, GM], F32, tag=f"r{tag}_{g}",
                                    name=f"r{tag}_{g}")
                nc.scalar.activation(r[:], rinv[:], AF.Sqrt)
                nc.vector.tensor_mul(scale[:, msl], g_t[:, msl], r[:])
                nc.vector.tensor_mul(bias[:, msl], mu[:], scale[:, msl])
                nc.vector.tensor_sub(bias[:, msl], bt_t[:, msl], bias[:, msl])

            # fc1-overlapped Sign pass using the hoisted pin/s1s staging pools:
            # p1d -> pin -> Sign -> s1stage -> s1d, on gpsimd DMA queues so
            # they bypass the busy sync/HWDGE queues and phase-pool aliasing.
            QS = 1024

            def sign_group(g):
                for mm in range(g * GM, (g + 1) * GM):
                    for q in range(BS // QS):
                        pin = pin_pool.tile([128, QS], F32, tag="pin",
                                            name=f"pin_{mm}_{q}")
                        nc.gpsimd.dma_start(
                            pin[:], p1d[mm, :, q * QS:(q + 1) * QS]
                        )
                        st = s1s_pool.tile([128, QS], F8, tag="s1s",
                                           name=f"s1s_{mm}_{q}")
                        nc.scalar.activation(
                            st[:], pin[:], AF.Sign,
                            bias=bias1[:, mm:mm + 1], scale=scale1[:, mm:mm + 1],
                        )
                        nc.gpsimd.dma_start(
                            s1d[mm, :, q * QS:(q + 1) * QS], st[:]
                        )

            # ================= Phase 1: fc1 + prelu + stats ==================
            with (
                tc.tile_pool(name="xt", bufs=1) as xt_pool,
                tc.tile_pool(name="w1p", bufs=2) as w1_pool,
                tc.tile_pool(name="p1t", bufs=3) as p1_pool,
                tc.tile_pool(name="scr1", bufs=2) as scr_pool,
            ):
                xt_t = xt_pool.tile([128, NB, KC1, 512], F16, tag="xt")
                for n in range(NB):
                    nc.sync.dma_start(xt_t[:, n, :, :], xT.ap()[:, n, :, :])
                for m in range(MT):
                    w1_t = w1_pool.tile([128, KC1, 128], F16, tag="w1")
                    nc.sync.dma_start(w1_t[:], w1.ap()[m])
                    for n in range(NB):
                        ps = ps_mm.tile([128, 512], F32, tag="mm")
                        for k in range(KC1):
                            nc.tensor.matmul(
                                ps[:], w1_t[:, k, :], xt_t[:, n, k, :],
                                start=(k == 0), stop=(k == KC1 - 1),
                            )
                        p1_t = p1_pool.tile([128, 512], F32, tag="p1")
                        nc.scalar.activation(
                            p1_t[:], ps[:], AF.Prelu, alpha=a1_t[:],
                            scale=1.0 / FSPLIT,
                            accum_out=sums1[:, m, n:n + 1],
                        )
                        scr = scr_pool.tile([128, 512], F16, tag="scr")
                        nc.vector.scalar_tensor_tensor(
                            scr[:], p1_t[:], 0.0, p1_t[:], ALU.add, ALU.mult,
                            accum_out=sq1[:, m, n:n + 1],
                        )
                        nc.sync.dma_start(
                            p1d[m, :, n * 512:(n + 1) * 512], p1_t[:]
                        )
                    if m % GM == GM - 1:
                        bn_group(sums1, sq1, cc_in1, cc_out1, g1_t, bt1_t,
                                 scale1, bias1, m // GM, "1")
                        sign_group(m // GM)

            # ================= Phase 2: fc2 + prelu + stats ==================
            with (
                tc.tile_pool(name="s1", bufs=2) as s1_pool,
                tc.tile_pool(name="w2p", bufs=4) as w2_pool,
                tc.tile_pool(name="p2t", bufs=4) as p2_pool,
                tc.tile_pool(name="sc2", bufs=3) as scr2_pool,
            ):
                for s in range(NSUP):
                    lo = s * SUPER
                    s1_t = s1_pool.tile([128, MT, SUPER], F8, tag="s1")
                    for k in range(MT):
                        nc.sync.dma_start(s1_t[:, k, :], s1d[k, :, lo:lo + SUPER])
                    for m in range(MT):
                        w2_t = w2_pool.tile([128, MT, 128], F8, tag="w2")
                        nc.sync.dma_start(w2_t[:], w2.ap()[m])
                        for j in range(NNS):
                            ps = ps_mm.tile([128, 512], F32, tag="mm")
                            for kk in range(MT // 2):
                                nc.tensor.matmul(
                                    ps[:], w2_t[:, 2 * kk:2 * kk + 2, :],
                                    s1_t[:, 2 * kk:2 * kk + 2,
                                         j * 512:(j + 1) * 512],
                                    start=(kk == 0), stop=(kk == MT // 2 - 1),
                                    perf_mode=mybir.MatmulPerfMode.DoubleRow,
                                )
                            n_g = s * NNS + j
                            p2_t = p2_pool.tile([128, 512], F16, tag="p2")
                            nc.scalar.activation(
                                p2_t[:], ps[:], AF.Prelu, alpha=a2_t[:],
                                accum_out=sums2[:, m, n_g:n_g + 1],
                            )
                            scr = scr2_pool.tile([128, 512], F16, tag="scr2")
                            nc.vector.scalar_tensor_tensor(
                                scr[:], p2_t[:], 0.0, p2_t[:], ALU.add, ALU.mult,
                                accum_out=sq2[:, m, n_g:n_g + 1],
                            )
                            nc.sync.dma_start(
                                p2d[m, :, lo + j * 512:lo + (j + 1) * 512],
                                p2_t[:],
                            )
                        if s == NSUP - 1 and m % GM == GM - 1:
                            bn_group(sums2, sq2, cc_in2, cc_out2, g2_t, bt2_t,
                                     scale2, bias2, m // GM, "2")

            # ================= Phase 3: BN2 + fc3 + log_softmax ==============
            with (
                tc.tile_pool(name="qp", bufs=12) as q_pool,
                tc.tile_pool(name="lg", bufs=2) as lg_pool,
                tc.tile_pool(name="ps3", bufs=2, space="PSUM") as ps3_pool,
                tc.tile_pool(name="pst", bufs=2, space="PSUM") as pst_pool,
                tc.tile_pool(name="sm", bufs=4) as sm_pool,
                tc.tile_pool(name="op", bufs=4) as out_pool,
            ):
                for n in range(NB):
                    pl = ps3_pool.tile([C, 512], F32, tag="pl")
                    for k in range(MT):
                        qin = q_pool.tile([128, 512], F16, tag="qin")
                        nc.gpsimd.dma_start(
                            qin[:], p2d[k, :, n * 512:(n + 1) * 512]
                        )
                        q = q_pool.tile([128, 512], F16, tag="q")
                        nc.vector.tensor_scalar(
                            q[:], qin[:], scale2[:, k:k + 1],
                            bias2[:, k:k + 1], ALU.mult, ALU.add,
                        )
                        nc.tensor.matmul(
                            pl[:], w3_t[:, k, :], q[:],
                            start=(k == 0), stop=(k == MT - 1),
                        )
                    lg = lg_pool.tile([C, 512], F32, tag="lg")
                    nc.scalar.activation(lg[:], pl[:], AF.Identity, bias=b3_t[:])
                    for j in range(4):
                        pt = pst_pool.tile([128, C], F32, tag="pt")
                        nc.tensor.transpose(
                            pt[:], lg[:, j * 128:(j + 1) * 128], eye_t[:]
                        )
                        mx = sm_pool.tile([128, 1], F32, tag="mx")
                        nc.vector.reduce_max(
                            mx[:], pt[:], axis=mybir.AxisListType.X, negate=True
                        )
                        ex = sm_pool.tile([128, C], F32, tag="ex")
                        se = sm_pool.tile([128, 1], F32, tag="se")
                        nc.scalar.activation(
                            ex[:], pt[:], AF.Exp, bias=mx[:], accum_out=se[:]
                        )
                        ln = sm_pool.tile([128, 1], F32, tag="ln")
                        nc.scalar.activation(ln[:], se[:], AF.Ln)
                        adj = sm_pool.tile([128, 1], F32, tag="adj")
                        nc.vector.tensor_sub(adj[:], mx[:], ln[:])
                        ot = out_pool.tile([128, C], F32, tag="ot")
                        nc.vector.tensor_scalar(
                            ot[:], pt[:], adj[:], None, ALU.add
                        )
                        nc.sync.dma_start(
                            out.ap()[n * 512 + j * 128:n * 512 + (j + 1) * 128, :],
                            ot[:],
                        )

            if debug:
                for nm, t in [("dbg_stats1", cats["cat1_0"]),
                              ("dbg_red1", cats["red1_0"]),
                              ("dbg_scale1", scale1), ("dbg_bias1", bias1),
                              ("dbg_red2", cats["red2_0"])]:
                    nc.sync.dma_start(dbg[nm].ap(), t[:])
                d1f = stats_pool.tile([128, 512], F32, tag="d1f")
                nc.sync.dma_start(d1f[:], p1d[0, :, 0:512])
                nc.sync.dma_start(dbg["dbg_p1"].ap(), d1f[:])
                d2 = stats_pool.tile([128, 512], F16, tag="d2")
                nc.sync.dma_start(d2[:], p2d[0, :, 0:512])
                d2f = stats_pool.tile([128, 512], F32, tag="d2f")
                nc.vector.tensor_copy(d2f[:], d2[:])
                nc.sync.dma_start(dbg["dbg_p2"].ap(), d2f[:])

    nc.compile()
    return nc


def prep_inputs(x, W1, b1, a1, g1, beta1, W2, a2, g2, beta2, W3, b3):
    """Host-side layout prep. Returns per-core in_maps."""
    x = np.ascontiguousarray(np.asarray(x, np.float32))
    W1 = np.asarray(W1, np.float32)
    b1 = np.asarray(b1, np.float32)
    W2 = np.asarray(W2, np.float32)
    W3 = np.asarray(W3, np.float32)
    b3 = np.asarray(b3, np.float32)

    # fc1 operands with bias folded in as contraction row 784 (rows 785+ zero).
    # fp16 hi/lo split with 2^11 scaling, packed along K:
    #   XF = [xh; xh; xl*S],  WF = [wh*S; wl*S; wh]  ->  psum = S * h1
    # where v = vh + vl exactly captures ~22 mantissa bits.  The bias row uses
    # x-side 32.0 / w-side b1/32 to keep w*S within fp16 range.
    S = np.float32(FSPLIT)
    xT_aug = np.zeros((D + 1, B), np.float32)
    xT_aug[0:D] = x.T
    xT_aug[D] = 32.0
    w1T_aug = np.zeros((D + 1, H1), np.float32)
    w1T_aug[0:D] = W1.T
    w1T_aug[D] = b1 / 32.0

    xh = xT_aug.astype(np.float16)
    xl = ((xT_aug - xh.astype(np.float32)) * S).astype(np.float16)
    wh = w1T_aug.astype(np.float16)
    whs = (w1T_aug * S).astype(np.float16)
    wls = ((w1T_aug - wh.astype(np.float32)) * S).astype(np.float16)
    KPAD = KC1 * 128
    A = D + 1
    xF = np.zeros((KPAD, B), np.float16)
    xF[0:A] = xh
    xF[A:2 * A] = xh
    xF[2 * A:2 * A + D] = xl[0:D]
    wF = np.zeros((KPAD, H1), np.float16)
    wF[0:A] = whs
    wF[A:2 * A] = wls
    wF[2 * A:2 * A + D] = wh[0:D]
    w1_blk = np.ascontiguousarray(
        wF.reshape(KC1, 128, MT, 128).transpose(2, 1, 0, 3)
    )

    sW2T = np.where(W2 >= 0, np.float32(1), np.float32(-1)).T
    w2_blk = np.ascontiguousarray(
        sW2T.reshape(MT, 128, MT, 128).transpose(2, 1, 0, 3)
    ).astype(ml_dtypes.float8_e4m3)

    w3_blk = np.ascontiguousarray(
        W3.T.reshape(MT, 128, C).transpose(1, 0, 2)
    ).astype(np.float16)

    def feat_layout(v):
        return np.ascontiguousarray(np.asarray(v, np.float32).reshape(MT, 128).T)

    shared = dict(
        w1=w1_blk, w2=w2_blk, w3=w3_blk,
        g1=feat_layout(g1), bt1=feat_layout(beta1),
        g2=feat_layout(g2), bt2=feat_layout(beta2),
        a1p=np.full((128, 1), np.float32(a1), np.float32),
        a2p=np.full((128, 1), np.float32(a2), np.float32),
        b3p=b3.reshape(C, 1).astype(np.float32),
        eye=np.eye(C, dtype=np.float32),
    )
    in_maps = []
    for c in range(NCORES):
        sl = xF[:, c * BS:(c + 1) * BS]
        xs = np.ascontiguousarray(
            sl.reshape(KC1, 128, NB, 512).transpose(1, 2, 0, 3)
        )
        in_maps.append(dict(shared, xT=xs))
    return in_maps


_NC_CACHE = {}


def run(inputs, debug=False, trace=False):
    key = (debug,)
    if key not in _NC_CACHE:
        _NC_CACHE[key] = build_program(debug=debug)
    nc = _NC_CACHE[key]
    in_maps = prep_inputs(**inputs)
    res = run_bass_kernel_spmd(
        nc, in_maps, core_ids=list(range(NCORES)), trace=trace
    )
    outs = np.concatenate([res.results[c]["out"] for c in range(NCORES)], axis=0)
    return outs, res


def kernel(**inputs):
    out, _ = run(inputs)
    return out
